# revision 1
# baseline (speedup 1.0000x reference)
"""Trainium2 Bass kernel for nn_GATMissingEmbedder (3-layer GAT, N=10000, E=320000).

SPMD across 8 NeuronCores. Host relabels nodes into degree-homogeneous
windows of 128 (same per-window slot count on every core). Edge slot
(window j, slot s, partition p) holds the s-th incoming edge of window
node p; holes point at a pad table row whose alpha_src is -200 (weight
exp(lrelu(-200+adst)) ~ e^-40 ~ 0, h-part zero).

Per layer: sharded stage A computes feat rows [h | a_src | a_dst | pad]
(h = act @ W; alpha columns folded into the weight matrix on the host;
biases are applied after aggregation, matching PyG GATConv). AllGather
builds the full table; dma_gather (4 SWDGE queues) pulls per-edge rows
slot-major so partition == destination; DVE/ACT compute
w = exp(leakyrelu(a_src[src] + a_dst[dst])) and scale the gathered h in
place; the segment softmax numerator/denominator accumulate in PSUM via
identity-weight matmuls; window finalize divides, adds bias, applies
ELU, and transposes activations for the next layer's stage A.
"""
import math
from contextlib import ExitStack

import ml_dtypes
import numpy as np

N = 10000
E = 320000
NCORES = 8
P = 128
NWIN = 10                  # windows per core
NDEV = NWIN * P            # 1280 owned (permuted) nodes per core
NV = NCORES * NDEV         # 10240 total permuted node slots
PADROW = NV                # table row used by hole slots
BMAX = 16                  # max slots per gather block
NGBUF = 8                  # gather tile buffers
CH = (256, 256, 128)       # h width per layer
NH = (4, 4, 1)             # heads per layer
ROWE = (384, 384, 256)     # table row elems (bf16) per layer
bf16 = ml_dtypes.bfloat16

_CACHE = {}


# ---------------------------------------------------------------- host prep
def _ablk(a):
    """[H, C] -> block-diagonal [H*C, H]."""
    H, C = a.shape
    out = np.zeros((H * C, H), np.float32)
    for h in range(H):
        out[h * C:(h + 1) * C, h] = a[h]
    return out


def _prep_graph(edge_index):
    src, dst = np.asarray(edge_index[0]), np.asarray(edge_index[1])
    deg = np.bincount(dst, minlength=N)
    order = np.argsort(-deg, kind="stable")          # rank -> orig node
    pid_of_node = np.full(N, -1, np.int64)
    ranks = np.arange(NV)
    rows = ranks // (P * NCORES)
    qs = ranks % (P * NCORES)
    cores = qs % NCORES
    ps = qs // NCORES
    pids = cores * NDEV + rows * P + ps
    real = ranks < N
    pid_of_node[order[ranks[real]]] = pids[real]
    m = np.zeros(NWIN, np.int64)
    for j in range(NWIN):
        rj = ranks[(rows == j) & real]
        m[j] = max(int(deg[order[rj]].max()) if len(rj) else 0, 1)
    pdst = pid_of_node[dst]
    psrc = pid_of_node[src]
    eorder = np.argsort(pdst, kind="stable")
    pdst_s, psrc_s = pdst[eorder], psrc[eorder]
    starts = np.searchsorted(pdst_s, np.arange(NV))
    ends = np.searchsorted(pdst_s, np.arange(NV) + 1)
    nslot = int(m.sum())
    idx_all = np.full((NCORES, nslot * P), PADROW, np.int64)
    base = np.concatenate([[0], np.cumsum(m)])
    for c in range(NCORES):
        for j in range(NWIN):
            for p in range(P):
                pid = c * NDEV + j * P + p
                s0, s1 = starts[pid], ends[pid]
                if s1 == s0:
                    continue
                sl = (base[j] + np.arange(s1 - s0)) * P + p
                idx_all[c, sl] = psrc_s[s0:s1]
    ni = nslot * P
    wrapped = np.zeros((NCORES, 128, ni // 16), np.int16)
    for c in range(NCORES):
        w16 = idx_all[c].reshape(ni // 16, 16).T.astype(np.int16)
        wrapped[c] = np.tile(w16, (8, 1))
    return pid_of_node, m, wrapped


def _prep_inputs(inputs):
    pid_of_node, m, wrapped = _prep_graph(inputs["edge_index"])
    w1 = np.asarray(inputs["w1"], np.float32)
    w2 = np.asarray(inputs["w2"], np.float32)
    w3 = np.asarray(inputs["w3"], np.float32)
    wx1 = np.concatenate(
        [w1, w1 @ _ablk(np.asarray(inputs["a_src1"], np.float32)),
         w1 @ _ablk(np.asarray(inputs["a_dst1"], np.float32))], axis=1)
    wx2 = np.concatenate(
        [w2, w2 @ _ablk(np.asarray(inputs["a_src2"], np.float32)),
         w2 @ _ablk(np.asarray(inputs["a_dst2"], np.float32))], axis=1)
    wx3 = np.concatenate(
        [w3, w3 @ _ablk(np.asarray(inputs["a_src3"], np.float32)),
         w3 @ _ablk(np.asarray(inputs["a_dst3"], np.float32))], axis=1)
    x = np.asarray(inputs["x"], np.float32).reshape(-1)
    xp = np.zeros(NV, np.float32)
    xp[pid_of_node] = x
    padrow1 = np.zeros((1, ROWE[0]), bf16)
    padrow1[0, 256:260] = bf16(-200.0)
    padrow3 = np.zeros((1, ROWE[2]), bf16)
    padrow3[0, 128] = bf16(-200.0)
    common = {
        "pw": np.asarray(inputs["proj_w"], np.float32).reshape(1, 128),
        "pb_rep": np.tile(np.asarray(inputs["proj_b"], np.float32).reshape(1, 128), (128, 1)),
        "w1x": wx1.astype(bf16),
        "w2x": wx2.reshape(2, 128, 264).astype(bf16),
        "w3x": wx3.reshape(2, 128, 130).astype(bf16),
        "b1_rep": np.tile(np.asarray(inputs["b1"], np.float32).reshape(1, 256), (128, 1)),
        "b2_rep": np.tile(np.asarray(inputs["b2"], np.float32).reshape(1, 256), (128, 1)),
        "b3_rep": np.tile(np.asarray(inputs["b3"], np.float32).reshape(1, 128), (128, 1)),
        "ident": np.eye(128, dtype=bf16),
        "padrow1": padrow1,
        "padrow3": padrow3,
    }
    in_maps = []
    for c in range(NCORES):
        d = dict(common)
        d["x_own"] = xp[c * NDEV:(c + 1) * NDEV].reshape(1, NDEV).copy()
        d["idxs"] = wrapped[c]
        in_maps.append(d)
    return pid_of_node, m, in_maps


# ---------------------------------------------------------------- plan
class Plan:
    """Per-engine op lists. One counting semaphore per engine: every op
    increments its engine's sem (DMA ops by 16), and cross/same-engine
    dependencies wait on recorded ordinals. This matches the in-order
    engines and keeps one update per instruction."""

    ENGSEM = {"vector": "vself", "scalar": "aself", "tensor": "tself",
              "sync": "sself"}
    STEP = {"vector": 1, "scalar": 1, "tensor": 1, "sync": 16}

    def __init__(self):
        self.ops = {e: [] for e in ("sync", "gpsimd", "tensor", "vector", "scalar")}
        self.count = {}

    def add(self, eng, fn, waits=(), extra_inc=None):
        """Returns the engine-sem value after this op completes."""
        incs = []
        after = None
        if eng in self.ENGSEM:
            s = self.ENGSEM[eng]
            step = self.STEP[eng]
            prev = self.count.get(s, 0)
            waits = list(waits)
            if eng in ("vector", "scalar"):
                waits.append((s, prev))          # same-engine in-order model
            incs.append((s, step))
            after = prev + step
        if extra_inc is not None:
            incs.append(extra_inc)
        self.ops[eng].append((fn, [w for w in waits if w[1] > 0], incs))
        for sem, k in incs:
            self.count[sem] = self.count.get(sem, 0) + k
        return after

    def n(self, sem):
        return self.count.get(sem, 0)


# ---------------------------------------------------------------- program
def build_program(m):
    import concourse.bacc as bacc
    import concourse.mybir as mybir
    from concourse.library_config import mlp

    f32, bft, i16 = mybir.dt.float32, mybir.dt.bfloat16, mybir.dt.int16
    Alu = mybir.AluOpType
    Act = mybir.ActivationFunctionType

    m = [int(v) for v in m]
    nslot = sum(m)
    ni = nslot * P
    nc = bacc.Bacc("TRN2", num_swdge_queues=4)

    x_own = nc.dram_tensor("x_own", [1, NDEV], f32, kind="ExternalInput")
    idxs = nc.dram_tensor("idxs", [128, ni // 16], i16, kind="ExternalInput")
    pw = nc.dram_tensor("pw", [1, 128], f32, kind="ExternalInput")
    pb_rep = nc.dram_tensor("pb_rep", [128, 128], f32, kind="ExternalInput")
    w1x = nc.dram_tensor("w1x", [128, 264], bft, kind="ExternalInput")
    w2x = nc.dram_tensor("w2x", [2, 128, 264], bft, kind="ExternalInput")
    w3x = nc.dram_tensor("w3x", [2, 128, 130], bft, kind="ExternalInput")
    b1_rep = nc.dram_tensor("b1_rep", [128, 256], f32, kind="ExternalInput")
    b2_rep = nc.dram_tensor("b2_rep", [128, 256], f32, kind="ExternalInput")
    b3_rep = nc.dram_tensor("b3_rep", [128, 128], f32, kind="ExternalInput")
    identd = nc.dram_tensor("ident", [128, 128], bft, kind="ExternalInput")
    padrow1 = nc.dram_tensor("padrow1", [1, ROWE[0]], bft, kind="ExternalInput")
    padrow3 = nc.dram_tensor("padrow3", [1, ROWE[2]], bft, kind="ExternalInput")
    out_d = nc.dram_tensor("out", [NDEV, 128], f32, kind="ExternalOutput")

    table1 = nc.dram_tensor("table1", [NV + 1, ROWE[0]], bft, addr_space="Shared")
    table2 = nc.dram_tensor("table2", [NV + 1, ROWE[1]], bft, addr_space="Shared")
    table3 = nc.dram_tensor("table3", [NV + 1, ROWE[2]], bft, addr_space="Shared")
    bounce = nc.dram_tensor("bounce", [NDEV, ROWE[0]], bft)
    bounce3 = nc.dram_tensor("bounce3", [NDEV, ROWE[2]], bft)
    tables = (table1, table2, table3)
    bounces = (bounce, bounce, bounce3)

    wblocks = []
    for j in range(NWIN):
        bl, s0 = [], 0
        while s0 < m[j]:
            bs = min(BMAX, m[j] - s0)
            bl.append((s0, bs))
            s0 += bs
        wblocks.append(bl)
    mbase = [0]
    for j in range(NWIN):
        mbase.append(mbase[-1] + m[j])

    plan = Plan()
    T = {}          # recorded ordinals: T[(kind, idx)] = engine-sem threshold
    G = lambda k, d=0: T.get(k, d)

    with ExitStack() as ctx:
        sb = lambda name, shape, dt: ctx.enter_context(nc.sbuf_tensor(name, shape, dt))
        psumt = lambda name, shape, dt: ctx.enter_context(nc.psum_tensor(name, shape, dt))

        idx_sb = sb("idx_sb", [128, ni // 16], i16)
        pw_sb = sb("pw_sb", [1, 128], f32)
        x_sb = sb("x_sb", [1, NDEV], f32)
        pb_sb = sb("pb_sb", [128, 128], f32)
        w1x_sb = sb("w1x_sb", [128, 264], bft)
        w2x_sb = sb("w2x_sb", [128, 2, 264], bft)
        w3x_sb = sb("w3x_sb", [128, 2, 130], bft)
        b_sb = [sb("b1_sb", [128, 256], f32), sb("b2_sb", [128, 256], f32),
                sb("b3_sb", [128, 128], f32)]
        ident = sb("ident_sb", [128, 128], bft)
        prow1 = sb("prow1", [1, ROWE[0]], bft)
        prow3 = sb("prow3", [1, ROWE[2]], bft)

        gA = sb("gA", [128, NGBUF, BMAX, ROWE[0]], bft)
        g3 = sb("g3", [128, NGBUF, BMAX, ROWE[2]], bft)
        lgt = sb("lgt", [128, 2, BMAX, 4], f32)
        ltA = sb("ltA", [128, 2, BMAX, 4], f32)
        ltB = sb("ltB", [128, 2, BMAX, 4], f32)
        adst_sb = [sb(f"adst{l}", [128, NWIN, 4], bft) for l in range(3)]
        act1 = sb("act1", [128, 2, 128], bft)
        a1t = sb("a1t", [128, NDEV], bft)
        a2t = sb("a2t", [128, 2, NDEV], bft)
        a3t = sb("a3t", [128, 2, NDEV], bft)
        actT = (a1t, a2t, a3t)
        rowt = sb("rowt", [128, 2, ROWE[0]], bft)
        rowt3 = sb("rowt3", [128, 2, ROWE[2]], bft)
        t1f = sb("t1f", [128, 128], f32)
        m1f = sb("m1f", [128, 128], f32)
        p1f = sb("p1f", [128, 128], f32)
        e1f = sb("e1f", [128, 128], f32)
        tf = sb("tf", [128, 256], f32)
        fm = sb("fm", [128, 256], f32)
        fp = sb("fp", [128, 256], f32)
        fe = sb("fe", [128, 256], f32)
        rcp = sb("rcp", [128, 4], f32)
        aout = sb("aout", [128, 2, 256], bft)
        outr = sb("outr", [128, 2, 128], f32)

        psA = psumt("psA", [128, 2, 512], f32)
        psW = psumt("psW", [128, 4, 512], f32)
        psT = psumt("psT", [128, 2, 128], bft)
        psP = psumt("psP", [128, 2, 128], f32)

        sems = {s: ctx.enter_context(nc.semaphore(f"s_{s}"))
                for s in ("vself", "aself", "tself", "sself", "cc")}
        gsem = [ctx.enter_context(nc.semaphore(f"gq{i}")) for i in range(NGBUF)]
        gcnt = [0] * NGBUF

        # ---------------- boot ----------------
        loads = [(idx_sb[:, :], idxs[:, :]), (pw_sb[:, :], pw[:, :]),
                 (x_sb[:, :], x_own[:, :]), (pb_sb[:, :], pb_rep[:, :]),
                 (w1x_sb[:, :], w1x[:, :]),
                 (w2x_sb[:, :, :], w2x.rearrange("a b c -> b a c")),
                 (w3x_sb[:, :, :], w3x.rearrange("a b c -> b a c")),
                 (b_sb[0][:, :], b1_rep[:, :]), (b_sb[1][:, :], b2_rep[:, :]),
                 (b_sb[2][:, :], b3_rep[:, :]), (ident[:, :], identd[:, :]),
                 (prow1[:, :], padrow1[:, :]), (prow3[:, :], padrow3[:, :])]
        for dst, srcap in loads:
            T["boot_loads"] = plan.add(
                "sync", lambda e, d=dst, s=srcap: e.dma_start(out=d, in_=s))
        for tb, pr, ne in ((table1, prow1, ROWE[0]), (table2, prow1, ROWE[1]),
                           (table3, prow3, ROWE[2])):
            T["boot"] = plan.add(
                "sync",
                lambda e, t=tb, p=pr, ne=ne: e.dma_start(out=t[NV:NV + 1, 0:ne],
                                                         in_=p[0:1, 0:ne]),
                waits=[("sself", T["boot_loads"])])
        BOOTS = T["boot"]
        plan.add("vector", lambda e: e.memset(rowt[:, :, :], 0.0))
        T["rowz"] = plan.add("vector", lambda e: e.memset(rowt3[:, :, :], 0.0))

        # ---------------- stage A ----------------
        def stageA_chunk(l, j):
            ck = l * NWIN + j
            rows = slice(j * 128, (j + 1) * 128)
            ncols = (264, 264, 130)[l]
            rowtile = rowt if l < 2 else rowt3
            rowe = ROWE[l]
            psum = psA[:, j % 2, 0:ncols]
            if l == 0:
                T[("p1", j)] = plan.add(
                    "tensor",
                    lambda e: e.matmul(psP[:, j % 2, :], x_sb[0:1, rows],
                                       pw_sb[0:1, :], start=True, stop=True),
                    waits=[("sself", BOOTS), ("vself", G(("e1a", j - 2)))])
                plan.add(
                    "vector",
                    lambda e: e.tensor_tensor(out=t1f[:, :], in0=psP[:, j % 2, :],
                                              in1=pb_sb[:, :], op=Alu.add),
                    waits=[("sself", BOOTS), ("tself", T[("p1", j)]),
                           ("aself", G(("x1", j - 1)))])
                plan.add(
                    "vector",
                    lambda e: e.tensor_scalar(out=m1f[:, :], in0=t1f[:, :],
                                              scalar1=0.0, scalar2=None, op0=Alu.min))
                T[("e1a", j)] = plan.add(
                    "vector",
                    lambda e: e.tensor_scalar(out=p1f[:, :], in0=t1f[:, :],
                                              scalar1=0.0, scalar2=-1.0,
                                              op0=Alu.max, op1=Alu.add))
                T[("x1", j)] = plan.add(
                    "scalar",
                    lambda e: e.activation(e1f[:, :], m1f[:, :], Act.Exp),
                    waits=[("vself", T[("e1a", j)]), ("vself", G(("e1", j - 1)))])
                T[("e1", j)] = plan.add(
                    "vector",
                    lambda e: e.tensor_tensor(out=act1[:, j % 2, :], in0=e1f[:, :],
                                              in1=p1f[:, :], op=Alu.add),
                    waits=[("aself", T[("x1", j)]), ("tself", G(("t1", j - 2)))])
                T[("t1", j)] = plan.add(
                    "tensor",
                    lambda e: e.transpose(psT[:, j % 2, :], act1[:, j % 2, :],
                                          ident[:, :]),
                    waits=[("vself", T[("e1", j)]), ("aself", G(("c1", j - 2)))])
                T[("c1", j)] = plan.add(
                    "scalar",
                    lambda e: e.activation(a1t[:, rows], psT[:, j % 2, :], Act.Copy),
                    waits=[("tself", T[("t1", j)])])
                T[("feat", ck)] = plan.add(
                    "tensor",
                    lambda e: e.matmul(psum, a1t[:, rows], w1x_sb[:, :],
                                       start=True, stop=True),
                    waits=[("aself", T[("c1", j)]),
                           ("aself", G(("rowcp", ck - 2))),
                           ("vself", G(("adst", ck - 2)))])
            else:
                at = actT[l]
                wsb = (None, w2x_sb, w3x_sb)[l]
                for k in range(2):
                    tk = plan.add(
                        "tensor",
                        lambda e, k=k, at=at, wsb=wsb, psum=psum, rows=rows,
                        ncols=ncols:
                        e.matmul(psum, at[:, k, rows], wsb[:, k, 0:ncols],
                                 start=(k == 0), stop=(k == 1)),
                        waits=([("aself", T[("atc", (l - 1) * NWIN + j)]),
                                ("aself", G(("rowcp", ck - 2))),
                                ("vself", G(("adst", ck - 2)))]
                               if k == 0 else []))
                T[("feat", ck)] = tk
            rw = [("tself", T[("feat", ck)]), ("sself", G(("row", ck - 2)))]
            if ck == 0:
                rw.append(("vself", T["rowz"]))
            T[("rowcp", ck)] = plan.add(
                "scalar",
                lambda e, rowtile=rowtile, ck=ck, ncols=ncols, psum=psum:
                e.activation(rowtile[:, ck % 2, 0:ncols], psum, Act.Copy),
                waits=rw)
            nh = NH[l]
            ac0 = (260, 260, 129)[l]
            T[("adst", ck)] = plan.add(
                "vector",
                lambda e, l=l, j=j, nh=nh, ac0=ac0:
                e.tensor_copy(out=adst_sb[l][:, j, 0:nh],
                              in_=psA[:, j % 2, ac0:ac0 + nh]),
                waits=[("tself", T[("feat", ck)])])
            bn = bounces[l]
            dw = [("aself", T[("rowcp", ck)])]
            if j == 0 and l >= 1:
                dw.append(("cc", l))
            T[("row", ck)] = plan.add(
                "sync",
                lambda e, bn=bn, rows=rows, rowtile=rowtile, ck=ck, rowe=rowe:
                e.dma_start(out=bn[rows, :], in_=rowtile[:, ck % 2, 0:rowe]),
                waits=dw)

        def allgather(l):
            plan.add("gpsimd",
                     lambda e, tb=tables[l], bn=bounces[l]:
                     e.collective_compute("AllGather", Alu.bypass,
                                          replica_groups=[list(range(NCORES))],
                                          ins=[bn[:, :]], outs=[tb[0:NV, :]]),
                     waits=[("sself", T[("row", l * NWIN + NWIN - 1)])],
                     extra_inc=("cc", 1))

        # ---------------- edge sweep ----------------
        qglob = [0]

        def edge_window(l, j):
            rowe = ROWE[l]
            nh = NH[l]
            ch = CH[l]
            rhsc = (260, 260, 129)[l]
            asc0 = (256, 256, 128)[l]
            gt = gA if l < 2 else g3
            wg = l * NWIN + j
            for (s0, bs) in wblocks[j]:
                gq = qglob[0]
                qglob[0] += 1
                slot = gq % NGBUF
                gtile = gt[:, slot]
                colbase = (mbase[j] + s0) * 8
                gwaits = [("cc", l + 1)]
                if gcnt[slot] > 0:
                    gwaits.append(("tself", G(("mm", gq - NGBUF))))
                gcnt[slot] += 1
                need = 16 * gcnt[slot]
                plan.add("gpsimd",
                         lambda e, gtile=gtile, colbase=colbase, bs=bs, slot=slot,
                         rowe=rowe, tb=tables[l]:
                         e.dma_gather(gtile[:, 0:bs, :], tb[:, :],
                                      idx_sb[:, colbase:colbase + bs * 8],
                                      bs * 128, bs * 128, rowe,
                                      single_packet=False, queue_num=gq % 4)
                         .then_inc(gsem[slot], 16),
                         waits=gwaits)
                lsl = lambda t, gq=gq, bs=bs, nh=nh: t[:, gq % 2, 0:bs, 0:nh]
                adst_ap = adst_sb[l][:, j:j + 1, 0:nh].to_broadcast([128, bs, nh])
                plan.add("vector",
                         lambda e, gtile=gtile, bs=bs, nh=nh, asc0=asc0, lsl=lsl,
                         adst_ap=adst_ap:
                         e.tensor_tensor(out=lsl(lgt),
                                         in0=gtile[:, 0:bs, asc0:asc0 + nh],
                                         in1=adst_ap, op=Alu.add),
                         waits=[(gsem[slot], need), ("aself", G(("ea", gq - 2))),
                                ("vself", G(("adstv", (l, j))))])
                T[("lg", gq)] = plan.n("vself")
                T[("ea", gq)] = plan.add(
                    "scalar",
                    lambda e, lsl=lsl: e.activation(lsl(ltA), lsl(lgt), Act.Exp),
                    waits=[("vself", T[("lg", gq)]),
                           ("vself", G(("wmax", gq - 2)))])
                T[("eb", gq)] = plan.add(
                    "scalar",
                    lambda e, lsl=lsl: e.activation(lsl(ltB), lsl(lgt), Act.Exp,
                                                    scale=0.2))
                T[("wmax", gq)] = plan.add(
                    "vector",
                    lambda e, gtile=gtile, bs=bs, nh=nh, asc0=asc0, lsl=lsl:
                    e.tensor_tensor(out=gtile[:, 0:bs, asc0:asc0 + nh],
                                    in0=lsl(ltA), in1=lsl(ltB), op=Alu.max),
                    waits=[("aself", T[("eb", gq)])])
                T[("gm", gq)] = plan.add(
                    "vector",
                    lambda e, gtile=gtile, bs=bs, nh=nh, ch=ch, asc0=asc0:
                    e.tensor_tensor(
                        out=gtile[:, 0:bs, 0:ch].rearrange(
                            "p b (h c) -> p b h c", h=nh),
                        in0=gtile[:, 0:bs, 0:ch].rearrange(
                            "p b (h c) -> p b h c", h=nh),
                        in1=gtile[:, 0:bs, asc0:asc0 + nh][:, :, :, None]
                        .to_broadcast([128, bs, nh, ch // nh]),
                        op=Alu.mult))
                for k in range(bs):
                    mw = []
                    if k == 0:
                        mw = [("vself", T[("gm", gq)])]
                        if s0 == 0:
                            mw.append(("vself", G(("fin", wg - 4))))
                    tk = plan.add(
                        "tensor",
                        lambda e, gtile=gtile, k=k, j=j, s0=s0, rhsc=rhsc, mj=m[j]:
                        e.matmul(psW[:, j % 4, 0:rhsc], ident[:, :],
                                 gtile[:, k, 0:rhsc],
                                 start=(s0 + k == 0), stop=(s0 + k == mj - 1)),
                        waits=mw)
                T[("mm", gq)] = tk
            T[("wlast", l, j)] = qglob[0] - 1

        # ---------------- finalize ----------------
        def finalize(l, j):
            wg = l * NWIN + j
            nh = NH[l]
            ch = CH[l]
            dn0 = (256, 256, 128)[l]
            lastq = T[("wlast", l, j)]
            plan.add("vector",
                     lambda e, j=j, nh=nh, dn0=dn0:
                     e.reciprocal(rcp[:, 0:nh], psW[:, j % 4, dn0:dn0 + nh]),
                     waits=[("tself", T[("mm", lastq)])])
            if l < 2:
                T[("fin", wg)] = plan.add(
                    "vector",
                    lambda e, j=j, nh=nh, ch=ch:
                    e.tensor_tensor(
                        out=tf[:, :].rearrange("p (h c) -> p h c", h=nh),
                        in0=psW[:, j % 4, 0:ch].rearrange("p (h c) -> p h c", h=nh),
                        in1=rcp[:, 0:nh][:, :, None].to_broadcast([128, nh, ch // nh]),
                        op=Alu.mult))
                plan.add("vector",
                         lambda e, l=l: e.tensor_tensor(out=tf[:, :], in0=tf[:, :],
                                                        in1=b_sb[l][:, :], op=Alu.add))
                plan.add("vector",
                         lambda e: e.tensor_scalar(out=fm[:, :], in0=tf[:, :],
                                                   scalar1=0.0, scalar2=None,
                                                   op0=Alu.min),
                         waits=[("aself", G(("felu", wg - 1)))])
                T[("fpre", wg)] = plan.add(
                    "vector",
                    lambda e: e.tensor_scalar(out=fp[:, :], in0=tf[:, :],
                                              scalar1=0.0, scalar2=-1.0,
                                              op0=Alu.max, op1=Alu.add))
                T[("felu", wg)] = plan.add(
                    "scalar",
                    lambda e: e.activation(fe[:, :], fm[:, :], Act.Exp),
                    waits=[("vself", T[("fpre", wg)])])
                T[("aoutv", wg)] = plan.add(
                    "vector",
                    lambda e, wg=wg: e.tensor_tensor(out=aout[:, wg % 2, :],
                                                     in0=fe[:, :], in1=fp[:, :],
                                                     op=Alu.add),
                    waits=[("aself", T[("felu", wg)]),
                           ("tself", G(("tr", wg - 2)))])
                at = actT[l + 1]
                for k in range(2):
                    tk = plan.add(
                        "tensor",
                        lambda e, k=k, wg=wg: e.transpose(
                            psT[:, k, :], aout[:, wg % 2, k * 128:(k + 1) * 128],
                            ident[:, :]),
                        waits=([("vself", T[("aoutv", wg)]),
                                ("aself", G(("atc", wg - 1)))] if k == 0 else []))
                T[("tr", wg)] = tk
                for k in range(2):
                    tk = plan.add(
                        "scalar",
                        lambda e, k=k, at=at, j=j: e.activation(
                            at[:, k, j * 128:(j + 1) * 128], psT[:, k, :], Act.Copy),
                        waits=([("tself", T[("tr", wg)])] if k == 0 else []))
                T[("atc", wg)] = tk
            else:
                T[("fin", wg)] = plan.add(
                    "vector",
                    lambda e, j=j: e.tensor_tensor(
                        out=outr[:, j % 2, :], in0=psW[:, j % 4, 0:128],
                        in1=rcp[:, 0:1].to_broadcast([128, 128]), op=Alu.mult),
                    waits=[("sself", G(("outd", j - 2)))])
                T[("outrow", j)] = plan.add(
                    "vector",
                    lambda e, j=j: e.tensor_tensor(out=outr[:, j % 2, :],
                                                   in0=outr[:, j % 2, :],
                                                   in1=b_sb[2][:, :], op=Alu.add))
                T[("outd", j)] = plan.add(
                    "sync",
                    lambda e, j=j: e.dma_start(out=out_d[j * 128:(j + 1) * 128, :],
                                               in_=outr[:, j % 2, :]),
                    waits=[("vself", T[("outrow", j)])])

        # ---------------- schedule ----------------
        for j in range(NWIN):
            stageA_chunk(0, j)
        allgather(0)
        for l in range(3):
            for j in range(NWIN):
                # adst for (l, j) was produced by stage A chunk (l, j)
                T[("adstv", (l, j))] = T[("adst", l * NWIN + j)]
                edge_window(l, j)
                finalize(l, j)
                if l < 2:
                    stageA_chunk(l + 1, j)
            if l < 2:
                allgather(l + 1)

        # ---------------- emit ----------------
        def run_engine(eng_obj, name):
            for fn, waits, incs in plan.ops[name]:
                for semname, cnt in waits:
                    h = sems[semname] if isinstance(semname, str) else semname
                    eng_obj.wait_ge(h, cnt)
                instr = fn(eng_obj)
                for semname, cnt in incs:
                    h = sems[semname] if isinstance(semname, str) else semname
                    instr = instr.then_inc(h, cnt)

        with nc.Block() as block:
            @block.gpsimd
            def _(gpsimd):
                gpsimd.load_library(mlp)
                run_engine(gpsimd, "gpsimd")

            @block.sync
            def _(sync):
                run_engine(sync, "sync")

            @block.tensor
            def _(tensor):
                run_engine(tensor, "tensor")

            @block.vector
            def _(vector):
                run_engine(vector, "vector")

            @block.scalar
            def _(scalar):
                run_engine(scalar, "scalar")

        nc.compile()
    return nc


# ---------------------------------------------------------------- entry
def kernel(**inputs):
    from concourse.bass_utils import run_bass_kernel_spmd

    pid_of_node, m, in_maps = _prep_inputs(inputs)
    key = tuple(int(v) for v in m)
    if key not in _CACHE:
        _CACHE[key] = build_program(m)
    nc = _CACHE[key]
    res = run_bass_kernel_spmd(nc, in_maps, list(range(NCORES)))
    outp = np.concatenate([res.results[c]["out"] for c in range(NCORES)], axis=0)
    return outp[pid_of_node].astype(np.float32)



# revision 2
# speedup vs baseline: 1.2606x; 1.2606x over previous
"""Trainium2 Bass kernel for nn_GATMissingEmbedder (3-layer GAT, N=10000, E=320000).

SPMD across 8 NeuronCores. Host relabels nodes into degree-homogeneous
windows of 128 (same per-window slot count on every core). Edge slot
(window j, slot s, partition p) holds the s-th incoming edge of window
node p; holes point at a pad table row whose alpha_src is -200 (weight
exp(lrelu(-200+adst)) ~ e^-40 ~ 0, h-part zero).

Per layer: sharded stage A computes feat rows [h | a_src | a_dst | pad]
(h = act @ W; alpha columns folded into the weight matrix on the host;
biases are applied after aggregation, matching PyG GATConv). AllGather
builds the full table; dma_gather (4 SWDGE queues) pulls per-edge rows
slot-major so partition == destination; DVE/ACT compute
w = exp(leakyrelu(a_src[src] + a_dst[dst])) and scale the gathered h in
place; the segment softmax numerator/denominator accumulate in PSUM via
identity-weight matmuls; window finalize divides, adds bias, applies
ELU, and transposes activations for the next layer's stage A.
"""
import math
from contextlib import ExitStack

import ml_dtypes
import numpy as np

N = 10000
E = 320000
NCORES = 8
P = 128
NWIN = 10                  # windows per core
NDEV = NWIN * P            # 1280 owned (permuted) nodes per core
NV = NCORES * NDEV         # 10240 total permuted node slots
PADROW = NV                # table row used by hole slots
BMAX = 16                  # max slots per gather block
NGBUF = 8                  # gather tile buffers
CH = (256, 256, 128)       # h width per layer
NH = (4, 4, 1)             # heads per layer
ROWE = (384, 384, 256)     # table row elems (bf16) per layer
bf16 = ml_dtypes.bfloat16

_CACHE = {}


# ---------------------------------------------------------------- host prep
def _ablk(a):
    """[H, C] -> block-diagonal [H*C, H]."""
    H, C = a.shape
    out = np.zeros((H * C, H), np.float32)
    for h in range(H):
        out[h * C:(h + 1) * C, h] = a[h]
    return out


def _prep_graph(edge_index):
    src, dst = np.asarray(edge_index[0]), np.asarray(edge_index[1])
    deg = np.bincount(dst, minlength=N)
    order = np.argsort(-deg, kind="stable")          # rank -> orig node
    pid_of_node = np.full(N, -1, np.int64)
    ranks = np.arange(NV)
    rows = ranks // (P * NCORES)
    qs = ranks % (P * NCORES)
    cores = qs % NCORES
    ps = qs // NCORES
    pids = cores * NDEV + rows * P + ps
    real = ranks < N
    pid_of_node[order[ranks[real]]] = pids[real]
    m = np.zeros(NWIN, np.int64)
    for j in range(NWIN):
        rj = ranks[(rows == j) & real]
        m[j] = max(int(deg[order[rj]].max()) if len(rj) else 0, 1)
    pdst = pid_of_node[dst]
    psrc = pid_of_node[src]
    eorder = np.argsort(pdst, kind="stable")
    pdst_s, psrc_s = pdst[eorder], psrc[eorder]
    starts = np.searchsorted(pdst_s, np.arange(NV))
    ends = np.searchsorted(pdst_s, np.arange(NV) + 1)
    nslot = int(m.sum())
    idx_all = np.full((NCORES, nslot * P), PADROW, np.int64)
    base = np.concatenate([[0], np.cumsum(m)])
    for c in range(NCORES):
        for j in range(NWIN):
            for p in range(P):
                pid = c * NDEV + j * P + p
                s0, s1 = starts[pid], ends[pid]
                if s1 == s0:
                    continue
                sl = (base[j] + np.arange(s1 - s0)) * P + p
                idx_all[c, sl] = psrc_s[s0:s1]
    ni = nslot * P
    wrapped = np.zeros((NCORES, 128, ni // 16), np.int16)
    for c in range(NCORES):
        w16 = idx_all[c].reshape(ni // 16, 16).T.astype(np.int16)
        wrapped[c] = np.tile(w16, (8, 1))
    return pid_of_node, m, wrapped


def _prep_inputs(inputs):
    pid_of_node, m, wrapped = _prep_graph(inputs["edge_index"])
    w1 = np.asarray(inputs["w1"], np.float32)
    w2 = np.asarray(inputs["w2"], np.float32)
    w3 = np.asarray(inputs["w3"], np.float32)
    wx1 = np.concatenate(
        [w1, w1 @ _ablk(np.asarray(inputs["a_src1"], np.float32)),
         w1 @ _ablk(np.asarray(inputs["a_dst1"], np.float32))], axis=1)
    wx2 = np.concatenate(
        [w2, w2 @ _ablk(np.asarray(inputs["a_src2"], np.float32)),
         w2 @ _ablk(np.asarray(inputs["a_dst2"], np.float32))], axis=1)
    wx3 = np.concatenate(
        [w3, w3 @ _ablk(np.asarray(inputs["a_src3"], np.float32)),
         w3 @ _ablk(np.asarray(inputs["a_dst3"], np.float32))], axis=1)
    x = np.asarray(inputs["x"], np.float32).reshape(-1)
    xp = np.zeros(NV, np.float32)
    xp[pid_of_node] = x
    padrow1 = np.zeros((1, ROWE[0]), bf16)
    padrow1[0, 256:260] = bf16(-200.0)
    padrow3 = np.zeros((1, ROWE[2]), bf16)
    padrow3[0, 128] = bf16(-200.0)
    common = {
        "pw": np.asarray(inputs["proj_w"], np.float32).reshape(1, 128),
        "pb_rep": np.tile(np.asarray(inputs["proj_b"], np.float32).reshape(1, 128), (128, 1)),
        "w1x": wx1.astype(bf16),
        "w2x": wx2.reshape(2, 128, 264).astype(bf16),
        "w3x": wx3.reshape(2, 128, 130).astype(bf16),
        "b1_rep": np.tile(np.asarray(inputs["b1"], np.float32).reshape(1, 256), (128, 1)),
        "b2_rep": np.tile(np.asarray(inputs["b2"], np.float32).reshape(1, 256), (128, 1)),
        "b3_rep": np.tile(np.asarray(inputs["b3"], np.float32).reshape(1, 128), (128, 1)),
        "ident": np.eye(128, dtype=bf16),
        "padrow1": padrow1,
        "padrow3": padrow3,
    }
    in_maps = []
    for c in range(NCORES):
        d = dict(common)
        d["x_own"] = xp[c * NDEV:(c + 1) * NDEV].reshape(1, NDEV).copy()
        d["idxs"] = wrapped[c]
        in_maps.append(d)
    return pid_of_node, m, in_maps


# ---------------------------------------------------------------- plan
class Plan:
    """Per-engine op lists. One counting semaphore per engine: every op
    increments its engine's sem (DMA ops by 16), and cross/same-engine
    dependencies wait on recorded ordinals. This matches the in-order
    engines and keeps one update per instruction."""

    ENGSEM = {"vector": "vself", "scalar": "aself", "tensor": "tself",
              "sync": "sself"}
    STEP = {"vector": 1, "scalar": 1, "tensor": 1, "sync": 16}

    def __init__(self):
        self.ops = {e: [] for e in ("sync", "gpsimd", "tensor", "vector", "scalar")}
        self.count = {}

    def add(self, eng, fn, waits=(), extra_inc=None):
        """Returns the engine-sem value after this op completes."""
        incs = []
        after = None
        if eng in self.ENGSEM:
            s = self.ENGSEM[eng]
            step = self.STEP[eng]
            prev = self.count.get(s, 0)
            waits = list(waits)
            if eng in ("vector", "scalar"):
                waits.append((s, prev))          # same-engine in-order model
            incs.append((s, step))
            after = prev + step
        if extra_inc is not None:
            incs.append(extra_inc)
        self.ops[eng].append((fn, [w for w in waits if w[1] > 0], incs))
        for sem, k in incs:
            self.count[sem] = self.count.get(sem, 0) + k
        return after

    def n(self, sem):
        return self.count.get(sem, 0)


# ---------------------------------------------------------------- program
def build_program(m):
    import concourse.bacc as bacc
    import concourse.mybir as mybir
    from concourse.library_config import mlp

    f32, bft, i16 = mybir.dt.float32, mybir.dt.bfloat16, mybir.dt.int16
    Alu = mybir.AluOpType
    Act = mybir.ActivationFunctionType

    m = [int(v) for v in m]
    nslot = sum(m)
    ni = nslot * P
    nc = bacc.Bacc("TRN2", num_swdge_queues=4)

    x_own = nc.dram_tensor("x_own", [1, NDEV], f32, kind="ExternalInput")
    idxs = nc.dram_tensor("idxs", [128, ni // 16], i16, kind="ExternalInput")
    pw = nc.dram_tensor("pw", [1, 128], f32, kind="ExternalInput")
    pb_rep = nc.dram_tensor("pb_rep", [128, 128], f32, kind="ExternalInput")
    w1x = nc.dram_tensor("w1x", [128, 264], bft, kind="ExternalInput")
    w2x = nc.dram_tensor("w2x", [2, 128, 264], bft, kind="ExternalInput")
    w3x = nc.dram_tensor("w3x", [2, 128, 130], bft, kind="ExternalInput")
    b1_rep = nc.dram_tensor("b1_rep", [128, 256], f32, kind="ExternalInput")
    b2_rep = nc.dram_tensor("b2_rep", [128, 256], f32, kind="ExternalInput")
    b3_rep = nc.dram_tensor("b3_rep", [128, 128], f32, kind="ExternalInput")
    identd = nc.dram_tensor("ident", [128, 128], bft, kind="ExternalInput")
    padrow1 = nc.dram_tensor("padrow1", [1, ROWE[0]], bft, kind="ExternalInput")
    padrow3 = nc.dram_tensor("padrow3", [1, ROWE[2]], bft, kind="ExternalInput")
    out_d = nc.dram_tensor("out", [NDEV, 128], f32, kind="ExternalOutput")

    table1 = nc.dram_tensor("table1", [NV + 1, ROWE[0]], bft, addr_space="Shared")
    table2 = nc.dram_tensor("table2", [NV + 1, ROWE[1]], bft, addr_space="Shared")
    table3 = nc.dram_tensor("table3", [NV + 1, ROWE[2]], bft, addr_space="Shared")
    bounce = nc.dram_tensor("bounce", [NDEV, ROWE[0]], bft)
    bounce3 = nc.dram_tensor("bounce3", [NDEV, ROWE[2]], bft)
    tables = (table1, table2, table3)
    bounces = (bounce, bounce, bounce3)

    wblocks = []
    for j in range(NWIN):
        bl, s0 = [], 0
        while s0 < m[j]:
            bs = min(BMAX, m[j] - s0)
            bl.append((s0, bs))
            s0 += bs
        wblocks.append(bl)
    mbase = [0]
    for j in range(NWIN):
        mbase.append(mbase[-1] + m[j])

    plan = Plan()
    T = {}          # recorded ordinals: T[(kind, idx)] = engine-sem threshold
    G = lambda k, d=0: T.get(k, d)

    with ExitStack() as ctx:
        sb = lambda name, shape, dt: ctx.enter_context(nc.sbuf_tensor(name, shape, dt))
        psumt = lambda name, shape, dt: ctx.enter_context(nc.psum_tensor(name, shape, dt))

        idx_sb = sb("idx_sb", [128, ni // 16], i16)
        pw_sb = sb("pw_sb", [1, 128], f32)
        x_sb = sb("x_sb", [1, NDEV], f32)
        pb_sb = sb("pb_sb", [128, 128], f32)
        w1x_sb = sb("w1x_sb", [128, 264], bft)
        w2x_sb = sb("w2x_sb", [128, 2, 264], bft)
        w3x_sb = sb("w3x_sb", [128, 2, 130], bft)
        b_sb = [sb("b1_sb", [128, 256], f32), sb("b2_sb", [128, 256], f32),
                sb("b3_sb", [128, 128], f32)]
        ident = sb("ident_sb", [128, 128], bft)
        prow1 = sb("prow1", [1, ROWE[0]], bft)
        prow3 = sb("prow3", [1, ROWE[2]], bft)

        gA = sb("gA", [128, NGBUF, BMAX, ROWE[0]], bft)
        g3 = sb("g3", [128, NGBUF, BMAX, ROWE[2]], bft)
        lgt = sb("lgt", [128, 2, BMAX, 4], f32)
        ltA = sb("ltA", [128, 2, BMAX, 4], f32)
        ltB = sb("ltB", [128, 2, BMAX, 4], f32)
        adst_sb = [sb(f"adst{l}", [128, NWIN, 4], bft) for l in range(3)]
        act1 = sb("act1", [128, 2, 128], bft)
        a1t = sb("a1t", [128, NDEV], bft)
        a2t = sb("a2t", [128, 2, NDEV], bft)
        a3t = sb("a3t", [128, 2, NDEV], bft)
        actT = (a1t, a2t, a3t)
        rowt = sb("rowt", [128, 2, ROWE[0]], bft)
        rowt3 = sb("rowt3", [128, 2, ROWE[2]], bft)
        t1f = sb("t1f", [128, 128], f32)
        m1f = sb("m1f", [128, 128], f32)
        p1f = sb("p1f", [128, 128], f32)
        e1f = sb("e1f", [128, 128], f32)
        tf = sb("tf", [128, 256], f32)
        fm = sb("fm", [128, 256], f32)
        fp = sb("fp", [128, 256], f32)
        fe = sb("fe", [128, 256], f32)
        rcp = sb("rcp", [128, 4], f32)
        aout = sb("aout", [128, 2, 256], bft)
        outr = sb("outr", [128, 2, 128], f32)

        psA = psumt("psA", [128, 2, 512], f32)
        psW = psumt("psW", [128, 4, 512], f32)
        psT = psumt("psT", [128, 2, 128], bft)
        psP = psumt("psP", [128, 2, 128], f32)

        sems = {s: ctx.enter_context(nc.semaphore(f"s_{s}"))
                for s in ("vself", "aself", "tself", "sself", "cc")}
        gsem = [ctx.enter_context(nc.semaphore(f"gq{i}")) for i in range(NGBUF)]
        gcnt = [0] * NGBUF

        # ---------------- boot ----------------
        loads = [(idx_sb[:, :], idxs[:, :]), (pw_sb[:, :], pw[:, :]),
                 (x_sb[:, :], x_own[:, :]), (pb_sb[:, :], pb_rep[:, :]),
                 (w1x_sb[:, :], w1x[:, :]),
                 (w2x_sb[:, :, :], w2x.rearrange("a b c -> b a c")),
                 (w3x_sb[:, :, :], w3x.rearrange("a b c -> b a c")),
                 (b_sb[0][:, :], b1_rep[:, :]), (b_sb[1][:, :], b2_rep[:, :]),
                 (b_sb[2][:, :], b3_rep[:, :]), (ident[:, :], identd[:, :]),
                 (prow1[:, :], padrow1[:, :]), (prow3[:, :], padrow3[:, :])]
        for dst, srcap in loads:
            T["boot_loads"] = plan.add(
                "sync", lambda e, d=dst, s=srcap: e.dma_start(out=d, in_=s))
        for tb, pr, ne in ((table1, prow1, ROWE[0]), (table2, prow1, ROWE[1]),
                           (table3, prow3, ROWE[2])):
            T["boot"] = plan.add(
                "sync",
                lambda e, t=tb, p=pr, ne=ne: e.dma_start(out=t[NV:NV + 1, 0:ne],
                                                         in_=p[0:1, 0:ne]),
                waits=[("sself", T["boot_loads"])])
        BOOTS = T["boot"]
        plan.add("vector", lambda e: e.memset(rowt[:, :, :], 0.0))
        T["rowz"] = plan.add("vector", lambda e: e.memset(rowt3[:, :, :], 0.0))

        # ---------------- stage A ----------------
        def stageA_chunk(l, j):
            ck = l * NWIN + j
            rows = slice(j * 128, (j + 1) * 128)
            ncols = (264, 264, 130)[l]
            rowtile = rowt if l < 2 else rowt3
            rowe = ROWE[l]
            psum = psA[:, j % 2, 0:ncols]
            if l == 0:
                T[("p1", j)] = plan.add(
                    "tensor",
                    lambda e: e.matmul(psP[:, j % 2, :], x_sb[0:1, rows],
                                       pw_sb[0:1, :], start=True, stop=True),
                    waits=[("sself", BOOTS), ("vself", G(("e1a", j - 2)))])
                plan.add(
                    "vector",
                    lambda e: e.tensor_tensor(out=t1f[:, :], in0=psP[:, j % 2, :],
                                              in1=pb_sb[:, :], op=Alu.add),
                    waits=[("sself", BOOTS), ("tself", T[("p1", j)]),
                           ("aself", G(("x1", j - 1)))])
                plan.add(
                    "vector",
                    lambda e: e.tensor_scalar(out=m1f[:, :], in0=t1f[:, :],
                                              scalar1=0.0, scalar2=None, op0=Alu.min))
                T[("e1a", j)] = plan.add(
                    "vector",
                    lambda e: e.tensor_scalar(out=p1f[:, :], in0=t1f[:, :],
                                              scalar1=0.0, scalar2=-1.0,
                                              op0=Alu.max, op1=Alu.add))
                T[("x1", j)] = plan.add(
                    "scalar",
                    lambda e: e.activation(e1f[:, :], m1f[:, :], Act.Exp),
                    waits=[("vself", T[("e1a", j)]), ("vself", G(("e1", j - 1)))])
                T[("e1", j)] = plan.add(
                    "vector",
                    lambda e: e.tensor_tensor(out=act1[:, j % 2, :], in0=e1f[:, :],
                                              in1=p1f[:, :], op=Alu.add),
                    waits=[("aself", T[("x1", j)]), ("tself", G(("t1", j - 2)))])
                T[("t1", j)] = plan.add(
                    "tensor",
                    lambda e: e.transpose(psT[:, j % 2, :], act1[:, j % 2, :],
                                          ident[:, :]),
                    waits=[("vself", T[("e1", j)]), ("aself", G(("c1", j - 2)))])
                T[("c1", j)] = plan.add(
                    "scalar",
                    lambda e: e.activation(a1t[:, rows], psT[:, j % 2, :], Act.Copy),
                    waits=[("tself", T[("t1", j)])])
                T[("feat", ck)] = plan.add(
                    "tensor",
                    lambda e: e.matmul(psum, a1t[:, rows], w1x_sb[:, :],
                                       start=True, stop=True),
                    waits=[("aself", T[("c1", j)]),
                           ("aself", G(("rowcp", ck - 2))),
                           ("vself", G(("adst", ck - 2)))])
            else:
                at = actT[l]
                wsb = (None, w2x_sb, w3x_sb)[l]
                for k in range(2):
                    tk = plan.add(
                        "tensor",
                        lambda e, k=k, at=at, wsb=wsb, psum=psum, rows=rows,
                        ncols=ncols:
                        e.matmul(psum, at[:, k, rows], wsb[:, k, 0:ncols],
                                 start=(k == 0), stop=(k == 1)),
                        waits=([("aself", T[("atc", (l - 1) * NWIN + j)]),
                                ("aself", G(("rowcp", ck - 2))),
                                ("vself", G(("adst", ck - 2)))]
                               if k == 0 else []))
                T[("feat", ck)] = tk
            rw = [("tself", T[("feat", ck)]), ("sself", G(("row", ck - 2)))]
            if ck == 0:
                rw.append(("vself", T["rowz"]))
            T[("rowcp", ck)] = plan.add(
                "scalar",
                lambda e, rowtile=rowtile, ck=ck, ncols=ncols, psum=psum:
                e.activation(rowtile[:, ck % 2, 0:ncols], psum, Act.Copy),
                waits=rw)
            nh = NH[l]
            ac0 = (260, 260, 129)[l]
            T[("adst", ck)] = plan.add(
                "vector",
                lambda e, l=l, j=j, nh=nh, ac0=ac0:
                e.tensor_copy(out=adst_sb[l][:, j, 0:nh],
                              in_=psA[:, j % 2, ac0:ac0 + nh]),
                waits=[("tself", T[("feat", ck)])])
            bn = bounces[l]
            dw = [("aself", T[("rowcp", ck)])]
            if j == 0 and l >= 1:
                dw.append(("cc", l))
            T[("row", ck)] = plan.add(
                "sync",
                lambda e, bn=bn, rows=rows, rowtile=rowtile, ck=ck, rowe=rowe:
                e.dma_start(out=bn[rows, :], in_=rowtile[:, ck % 2, 0:rowe]),
                waits=dw)

        def allgather(l):
            plan.add("gpsimd",
                     lambda e, tb=tables[l], bn=bounces[l]:
                     e.collective_compute("AllGather", Alu.bypass,
                                          replica_groups=[list(range(NCORES))],
                                          ins=[bn[:, :]], outs=[tb[0:NV, :]]),
                     waits=[("sself", T[("row", l * NWIN + NWIN - 1)])],
                     extra_inc=("cc", 1))

        # ---------------- edge sweep ----------------
        qglob = [0]

        def edge_window(l, j):
            rowe = ROWE[l]
            nh = NH[l]
            ch = CH[l]
            rhsc = (260, 260, 129)[l]
            asc0 = (256, 256, 128)[l]
            gt = gA if l < 2 else g3
            wg = l * NWIN + j
            for (s0, bs) in wblocks[j]:
                gq = qglob[0]
                qglob[0] += 1
                slot = gq % NGBUF
                gtile = gt[:, slot]
                colbase = (mbase[j] + s0) * 8
                gwaits = [("cc", l + 1)]
                if gcnt[slot] > 0:
                    gwaits.append(("tself", G(("mm", gq - NGBUF))))
                gcnt[slot] += 1
                need = 16 * gcnt[slot]
                plan.add("gpsimd",
                         lambda e, gtile=gtile, colbase=colbase, bs=bs, slot=slot,
                         rowe=rowe, tb=tables[l], gq=gq:
                         e.dma_gather(gtile[:, 0:bs, :], tb[:, :],
                                      idx_sb[:, colbase:colbase + bs * 8],
                                      bs * 128, bs * 128, rowe,
                                      single_packet=False, queue_num=gq % 4)
                         .then_inc(gsem[slot], 16),
                         waits=gwaits)
                lsl = lambda t, gq=gq, bs=bs, nh=nh: t[:, gq % 2, 0:bs, 0:nh]
                adst_ap = adst_sb[l][:, j:j + 1, 0:nh].to_broadcast([128, bs, nh])
                plan.add("vector",
                         lambda e, gtile=gtile, bs=bs, nh=nh, asc0=asc0, lsl=lsl,
                         adst_ap=adst_ap:
                         e.tensor_tensor(out=lsl(lgt),
                                         in0=gtile[:, 0:bs, asc0:asc0 + nh],
                                         in1=adst_ap, op=Alu.add),
                         waits=[(gsem[slot], need), ("aself", G(("ea", gq - 2))),
                                ("vself", G(("adstv", (l, j))))])
                T[("lg", gq)] = plan.n("vself")
                T[("ea", gq)] = plan.add(
                    "scalar",
                    lambda e, lsl=lsl: e.activation(lsl(ltA), lsl(lgt), Act.Exp),
                    waits=[("vself", T[("lg", gq)]),
                           ("vself", G(("wmax", gq - 2)))])
                T[("eb", gq)] = plan.add(
                    "scalar",
                    lambda e, lsl=lsl: e.activation(lsl(ltB), lsl(lgt), Act.Exp,
                                                    scale=0.2))
                T[("wmax", gq)] = plan.add(
                    "vector",
                    lambda e, gtile=gtile, bs=bs, nh=nh, asc0=asc0, lsl=lsl:
                    e.tensor_tensor(out=gtile[:, 0:bs, asc0:asc0 + nh],
                                    in0=lsl(ltA), in1=lsl(ltB), op=Alu.max),
                    waits=[("aself", T[("eb", gq)])])
                T[("gm", gq)] = plan.add(
                    "vector",
                    lambda e, gtile=gtile, bs=bs, nh=nh, ch=ch, asc0=asc0:
                    e.tensor_tensor(
                        out=gtile[:, 0:bs, 0:ch].rearrange(
                            "p b (h c) -> p b h c", h=nh),
                        in0=gtile[:, 0:bs, 0:ch].rearrange(
                            "p b (h c) -> p b h c", h=nh),
                        in1=gtile[:, 0:bs, asc0:asc0 + nh][:, :, :, None]
                        .to_broadcast([128, bs, nh, ch // nh]),
                        op=Alu.mult))
                for k in range(bs):
                    mw = []
                    if k == 0:
                        mw = [("vself", T[("gm", gq)])]
                        if s0 == 0:
                            mw.append(("vself", G(("fin", wg - 4))))
                    tk = plan.add(
                        "tensor",
                        lambda e, gtile=gtile, k=k, j=j, s0=s0, rhsc=rhsc, mj=m[j]:
                        e.matmul(psW[:, j % 4, 0:rhsc], ident[:, :],
                                 gtile[:, k, 0:rhsc],
                                 start=(s0 + k == 0), stop=(s0 + k == mj - 1)),
                        waits=mw)
                T[("mm", gq)] = tk
            T[("wlast", l, j)] = qglob[0] - 1

        # ---------------- finalize ----------------
        def finalize(l, j):
            wg = l * NWIN + j
            nh = NH[l]
            ch = CH[l]
            dn0 = (256, 256, 128)[l]
            lastq = T[("wlast", l, j)]
            plan.add("vector",
                     lambda e, j=j, nh=nh, dn0=dn0:
                     e.reciprocal(rcp[:, 0:nh], psW[:, j % 4, dn0:dn0 + nh]),
                     waits=[("tself", T[("mm", lastq)])])
            if l < 2:
                T[("fin", wg)] = plan.add(
                    "vector",
                    lambda e, j=j, nh=nh, ch=ch:
                    e.tensor_tensor(
                        out=tf[:, :].rearrange("p (h c) -> p h c", h=nh),
                        in0=psW[:, j % 4, 0:ch].rearrange("p (h c) -> p h c", h=nh),
                        in1=rcp[:, 0:nh][:, :, None].to_broadcast([128, nh, ch // nh]),
                        op=Alu.mult))
                plan.add("vector",
                         lambda e, l=l: e.tensor_tensor(out=tf[:, :], in0=tf[:, :],
                                                        in1=b_sb[l][:, :], op=Alu.add))
                plan.add("vector",
                         lambda e: e.tensor_scalar(out=fm[:, :], in0=tf[:, :],
                                                   scalar1=0.0, scalar2=None,
                                                   op0=Alu.min),
                         waits=[("aself", G(("felu", wg - 1)))])
                T[("fpre", wg)] = plan.add(
                    "vector",
                    lambda e: e.tensor_scalar(out=fp[:, :], in0=tf[:, :],
                                              scalar1=0.0, scalar2=-1.0,
                                              op0=Alu.max, op1=Alu.add))
                T[("felu", wg)] = plan.add(
                    "scalar",
                    lambda e: e.activation(fe[:, :], fm[:, :], Act.Exp),
                    waits=[("vself", T[("fpre", wg)])])
                T[("aoutv", wg)] = plan.add(
                    "vector",
                    lambda e, wg=wg: e.tensor_tensor(out=aout[:, wg % 2, :],
                                                     in0=fe[:, :], in1=fp[:, :],
                                                     op=Alu.add),
                    waits=[("aself", T[("felu", wg)]),
                           ("tself", G(("tr", wg - 2)))])
                at = actT[l + 1]
                for k in range(2):
                    tk = plan.add(
                        "tensor",
                        lambda e, k=k, wg=wg: e.transpose(
                            psT[:, k, :], aout[:, wg % 2, k * 128:(k + 1) * 128],
                            ident[:, :]),
                        waits=([("vself", T[("aoutv", wg)]),
                                ("aself", G(("atc", wg - 1)))] if k == 0 else []))
                T[("tr", wg)] = tk
                for k in range(2):
                    tk = plan.add(
                        "scalar",
                        lambda e, k=k, at=at, j=j: e.activation(
                            at[:, k, j * 128:(j + 1) * 128], psT[:, k, :], Act.Copy),
                        waits=([("tself", T[("tr", wg)])] if k == 0 else []))
                T[("atc", wg)] = tk
            else:
                T[("fin", wg)] = plan.add(
                    "vector",
                    lambda e, j=j: e.tensor_tensor(
                        out=outr[:, j % 2, :], in0=psW[:, j % 4, 0:128],
                        in1=rcp[:, 0:1].to_broadcast([128, 128]), op=Alu.mult),
                    waits=[("sself", G(("outd", j - 2)))])
                T[("outrow", j)] = plan.add(
                    "vector",
                    lambda e, j=j: e.tensor_tensor(out=outr[:, j % 2, :],
                                                   in0=outr[:, j % 2, :],
                                                   in1=b_sb[2][:, :], op=Alu.add))
                T[("outd", j)] = plan.add(
                    "sync",
                    lambda e, j=j: e.dma_start(out=out_d[j * 128:(j + 1) * 128, :],
                                               in_=outr[:, j % 2, :]),
                    waits=[("vself", T[("outrow", j)])])

        # ---------------- schedule ----------------
        for j in range(NWIN):
            stageA_chunk(0, j)
        allgather(0)
        for l in range(3):
            for j in range(NWIN):
                # adst for (l, j) was produced by stage A chunk (l, j)
                T[("adstv", (l, j))] = T[("adst", l * NWIN + j)]
                edge_window(l, j)
                finalize(l, j)
                if l < 2:
                    stageA_chunk(l + 1, j)
            if l < 2:
                allgather(l + 1)

        # ---------------- emit ----------------
        def run_engine(eng_obj, name):
            for fn, waits, incs in plan.ops[name]:
                for semname, cnt in waits:
                    h = sems[semname] if isinstance(semname, str) else semname
                    eng_obj.wait_ge(h, cnt)
                instr = fn(eng_obj)
                for semname, cnt in incs:
                    h = sems[semname] if isinstance(semname, str) else semname
                    instr = instr.then_inc(h, cnt)

        with nc.Block() as block:
            @block.gpsimd
            def _(gpsimd):
                gpsimd.load_library(mlp)
                run_engine(gpsimd, "gpsimd")

            @block.sync
            def _(sync):
                run_engine(sync, "sync")

            @block.tensor
            def _(tensor):
                run_engine(tensor, "tensor")

            @block.vector
            def _(vector):
                run_engine(vector, "vector")

            @block.scalar
            def _(scalar):
                run_engine(scalar, "scalar")

        nc.compile()
    return nc


# ---------------------------------------------------------------- entry
def kernel(**inputs):
    from concourse.bass_utils import run_bass_kernel_spmd

    pid_of_node, m, in_maps = _prep_inputs(inputs)
    key = tuple(int(v) for v in m)
    if key not in _CACHE:
        _CACHE[key] = build_program(m)
    nc = _CACHE[key]
    res = run_bass_kernel_spmd(nc, in_maps, list(range(NCORES)))
    outp = np.concatenate([res.results[c]["out"] for c in range(NCORES)], axis=0)
    return outp[pid_of_node].astype(np.float32)



# revision 3
# speedup vs baseline: 1.2652x; 1.0037x over previous
"""Trainium2 Bass kernel for nn_GATMissingEmbedder (3-layer GAT, N=10000, E=320000).

SPMD across 8 NeuronCores. Host relabels nodes into degree-homogeneous
windows of 128 (same per-window slot count on every core). Edge slot
(window j, slot s, partition p) holds the s-th incoming edge of window
node p; holes point at a pad table row whose alpha_src is -200 (weight
exp(lrelu(-200+adst)) ~ e^-40 ~ 0, h-part zero).

Per layer: sharded stage A computes feat rows [h | a_src | a_dst | pad]
(h = act @ W; alpha columns folded into the weight matrix on the host;
biases are applied after aggregation, matching PyG GATConv). AllGather
builds the full table; dma_gather (4 SWDGE queues) pulls per-edge rows
slot-major so partition == destination; DVE/ACT compute
w = exp(leakyrelu(a_src[src] + a_dst[dst])) and scale the gathered h in
place; the segment softmax numerator/denominator accumulate in PSUM via
identity-weight matmuls; window finalize divides, adds bias, applies
ELU, and transposes activations for the next layer's stage A.
"""
import math
from contextlib import ExitStack

import ml_dtypes
import numpy as np

N = 10000
E = 320000
NCORES = 8
P = 128
NWIN = 10                  # windows per core
NDEV = NWIN * P            # 1280 owned (permuted) nodes per core
NV = NCORES * NDEV         # 10240 total permuted node slots
PADROW = NV                # table row used by hole slots
BMAX = 8                   # max slots per gather block
NGBUF = 16                 # gather tile buffers
CH = (256, 256, 128)       # h width per layer
NH = (4, 4, 1)             # heads per layer
ROWE = (384, 384, 256)     # table row elems (bf16) per layer
bf16 = ml_dtypes.bfloat16

_CACHE = {}


# ---------------------------------------------------------------- host prep
def _ablk(a):
    """[H, C] -> block-diagonal [H*C, H]."""
    H, C = a.shape
    out = np.zeros((H * C, H), np.float32)
    for h in range(H):
        out[h * C:(h + 1) * C, h] = a[h]
    return out


def _prep_graph(edge_index):
    src, dst = np.asarray(edge_index[0]), np.asarray(edge_index[1])
    deg = np.bincount(dst, minlength=N)
    order = np.argsort(-deg, kind="stable")          # rank -> orig node
    pid_of_node = np.full(N, -1, np.int64)
    ranks = np.arange(NV)
    rows = ranks // (P * NCORES)
    qs = ranks % (P * NCORES)
    cores = qs % NCORES
    ps = qs // NCORES
    pids = cores * NDEV + rows * P + ps
    real = ranks < N
    pid_of_node[order[ranks[real]]] = pids[real]
    m = np.zeros(NWIN, np.int64)
    for j in range(NWIN):
        rj = ranks[(rows == j) & real]
        m[j] = max(int(deg[order[rj]].max()) if len(rj) else 0, 1)
    pdst = pid_of_node[dst]
    psrc = pid_of_node[src]
    eorder = np.argsort(pdst, kind="stable")
    pdst_s, psrc_s = pdst[eorder], psrc[eorder]
    starts = np.searchsorted(pdst_s, np.arange(NV))
    ends = np.searchsorted(pdst_s, np.arange(NV) + 1)
    nslot = int(m.sum())
    idx_all = np.full((NCORES, nslot * P), PADROW, np.int64)
    base = np.concatenate([[0], np.cumsum(m)])
    for c in range(NCORES):
        for j in range(NWIN):
            for p in range(P):
                pid = c * NDEV + j * P + p
                s0, s1 = starts[pid], ends[pid]
                if s1 == s0:
                    continue
                sl = (base[j] + np.arange(s1 - s0)) * P + p
                idx_all[c, sl] = psrc_s[s0:s1]
    ni = nslot * P
    wrapped = np.zeros((NCORES, 128, ni // 16), np.int16)
    for c in range(NCORES):
        w16 = idx_all[c].reshape(ni // 16, 16).T.astype(np.int16)
        wrapped[c] = np.tile(w16, (8, 1))
    return pid_of_node, m, wrapped


def _prep_inputs(inputs):
    pid_of_node, m, wrapped = _prep_graph(inputs["edge_index"])
    w1 = np.asarray(inputs["w1"], np.float32)
    w2 = np.asarray(inputs["w2"], np.float32)
    w3 = np.asarray(inputs["w3"], np.float32)
    wx1 = np.concatenate(
        [w1, w1 @ _ablk(np.asarray(inputs["a_src1"], np.float32)),
         w1 @ _ablk(np.asarray(inputs["a_dst1"], np.float32))], axis=1)
    wx2 = np.concatenate(
        [w2, w2 @ _ablk(np.asarray(inputs["a_src2"], np.float32)),
         w2 @ _ablk(np.asarray(inputs["a_dst2"], np.float32))], axis=1)
    wx3 = np.concatenate(
        [w3, w3 @ _ablk(np.asarray(inputs["a_src3"], np.float32)),
         w3 @ _ablk(np.asarray(inputs["a_dst3"], np.float32))], axis=1)
    x = np.asarray(inputs["x"], np.float32).reshape(-1)
    xp = np.zeros(NV, np.float32)
    xp[pid_of_node] = x
    padrow1 = np.zeros((1, ROWE[0]), bf16)
    padrow1[0, 256:260] = bf16(-200.0)
    padrow3 = np.zeros((1, ROWE[2]), bf16)
    padrow3[0, 128] = bf16(-200.0)
    common = {
        "pw": np.asarray(inputs["proj_w"], np.float32).reshape(1, 128),
        "pb_rep": np.tile(np.asarray(inputs["proj_b"], np.float32).reshape(1, 128), (128, 1)),
        "w1x": wx1.astype(bf16),
        "w2x": wx2.reshape(2, 128, 264).astype(bf16),
        "w3x": wx3.reshape(2, 128, 130).astype(bf16),
        "b1_rep": np.tile(np.asarray(inputs["b1"], np.float32).reshape(1, 256), (128, 1)),
        "b2_rep": np.tile(np.asarray(inputs["b2"], np.float32).reshape(1, 256), (128, 1)),
        "b3_rep": np.tile(np.asarray(inputs["b3"], np.float32).reshape(1, 128), (128, 1)),
        "ident": np.eye(128, dtype=bf16),
        "padrow1": padrow1,
        "padrow3": padrow3,
    }
    in_maps = []
    for c in range(NCORES):
        d = dict(common)
        d["x_own"] = xp[c * NDEV:(c + 1) * NDEV].reshape(1, NDEV).copy()
        d["idxs"] = wrapped[c]
        in_maps.append(d)
    return pid_of_node, m, in_maps


# ---------------------------------------------------------------- plan
class Plan:
    """Per-engine op lists. One counting semaphore per engine: every op
    increments its engine's sem (DMA ops by 16), and cross/same-engine
    dependencies wait on recorded ordinals. This matches the in-order
    engines and keeps one update per instruction."""

    ENGSEM = {"vector": "vself", "scalar": "aself", "tensor": "tself",
              "sync": "sself"}
    STEP = {"vector": 1, "scalar": 1, "tensor": 1, "sync": 16}

    def __init__(self):
        self.ops = {e: [] for e in ("sync", "gpsimd", "tensor", "vector", "scalar")}
        self.count = {}

    def add(self, eng, fn, waits=(), extra_inc=None):
        """Returns the engine-sem value after this op completes."""
        incs = []
        after = None
        if eng in self.ENGSEM:
            s = self.ENGSEM[eng]
            step = self.STEP[eng]
            prev = self.count.get(s, 0)
            waits = list(waits)
            if eng in ("vector", "scalar"):
                waits.append((s, prev))          # same-engine in-order model
            incs.append((s, step))
            after = prev + step
        if extra_inc is not None:
            incs.append(extra_inc)
        self.ops[eng].append((fn, [w for w in waits if w[1] > 0], incs))
        for sem, k in incs:
            self.count[sem] = self.count.get(sem, 0) + k
        return after

    def n(self, sem):
        return self.count.get(sem, 0)


# ---------------------------------------------------------------- program
def build_program(m):
    import concourse.bacc as bacc
    import concourse.mybir as mybir
    from concourse.library_config import mlp

    f32, bft, i16 = mybir.dt.float32, mybir.dt.bfloat16, mybir.dt.int16
    Alu = mybir.AluOpType
    Act = mybir.ActivationFunctionType

    m = [int(v) for v in m]
    nslot = sum(m)
    ni = nslot * P
    nc = bacc.Bacc("TRN2", num_swdge_queues=4)

    x_own = nc.dram_tensor("x_own", [1, NDEV], f32, kind="ExternalInput")
    idxs = nc.dram_tensor("idxs", [128, ni // 16], i16, kind="ExternalInput")
    pw = nc.dram_tensor("pw", [1, 128], f32, kind="ExternalInput")
    pb_rep = nc.dram_tensor("pb_rep", [128, 128], f32, kind="ExternalInput")
    w1x = nc.dram_tensor("w1x", [128, 264], bft, kind="ExternalInput")
    w2x = nc.dram_tensor("w2x", [2, 128, 264], bft, kind="ExternalInput")
    w3x = nc.dram_tensor("w3x", [2, 128, 130], bft, kind="ExternalInput")
    b1_rep = nc.dram_tensor("b1_rep", [128, 256], f32, kind="ExternalInput")
    b2_rep = nc.dram_tensor("b2_rep", [128, 256], f32, kind="ExternalInput")
    b3_rep = nc.dram_tensor("b3_rep", [128, 128], f32, kind="ExternalInput")
    identd = nc.dram_tensor("ident", [128, 128], bft, kind="ExternalInput")
    padrow1 = nc.dram_tensor("padrow1", [1, ROWE[0]], bft, kind="ExternalInput")
    padrow3 = nc.dram_tensor("padrow3", [1, ROWE[2]], bft, kind="ExternalInput")
    out_d = nc.dram_tensor("out", [NDEV, 128], f32, kind="ExternalOutput")

    table1 = nc.dram_tensor("table1", [NV + 1, ROWE[0]], bft, addr_space="Shared")
    table2 = nc.dram_tensor("table2", [NV + 1, ROWE[1]], bft, addr_space="Shared")
    table3 = nc.dram_tensor("table3", [NV + 1, ROWE[2]], bft, addr_space="Shared")
    bounce = nc.dram_tensor("bounce", [NDEV, ROWE[0]], bft)
    bounce3 = nc.dram_tensor("bounce3", [NDEV, ROWE[2]], bft)
    tables = (table1, table2, table3)
    bounces = (bounce, bounce, bounce3)

    wblocks = []
    for j in range(NWIN):
        bl, s0 = [], 0
        while s0 < m[j]:
            bs = min(BMAX, m[j] - s0)
            bl.append((s0, bs))
            s0 += bs
        wblocks.append(bl)
    mbase = [0]
    for j in range(NWIN):
        mbase.append(mbase[-1] + m[j])

    plan = Plan()
    T = {}          # recorded ordinals: T[(kind, idx)] = engine-sem threshold
    G = lambda k, d=0: T.get(k, d)

    with ExitStack() as ctx:
        sb = lambda name, shape, dt: ctx.enter_context(nc.sbuf_tensor(name, shape, dt))
        psumt = lambda name, shape, dt: ctx.enter_context(nc.psum_tensor(name, shape, dt))

        idx_sb = sb("idx_sb", [128, ni // 16], i16)
        pw_sb = sb("pw_sb", [1, 128], f32)
        x_sb = sb("x_sb", [1, NDEV], f32)
        pb_sb = sb("pb_sb", [128, 128], f32)
        w1x_sb = sb("w1x_sb", [128, 264], bft)
        w2x_sb = sb("w2x_sb", [128, 2, 264], bft)
        w3x_sb = sb("w3x_sb", [128, 2, 130], bft)
        b_sb = [sb("b1_sb", [128, 256], f32), sb("b2_sb", [128, 256], f32),
                sb("b3_sb", [128, 128], f32)]
        ident = sb("ident_sb", [128, 128], bft)
        prow1 = sb("prow1", [1, ROWE[0]], bft)
        prow3 = sb("prow3", [1, ROWE[2]], bft)

        gA = sb("gA", [128, NGBUF, BMAX, ROWE[0]], bft)
        g3 = sb("g3", [128, NGBUF, BMAX, ROWE[2]], bft)
        lgt = sb("lgt", [128, 2, BMAX, 4], f32)
        ltA = sb("ltA", [128, 2, BMAX, 4], f32)
        ltB = sb("ltB", [128, 2, BMAX, 4], f32)
        adst_sb = [sb(f"adst{l}", [128, NWIN, 4], bft) for l in range(3)]
        act1 = sb("act1", [128, 2, 128], bft)
        a1t = sb("a1t", [128, NDEV], bft)
        a2t = sb("a2t", [128, 2, NDEV], bft)
        a3t = sb("a3t", [128, 2, NDEV], bft)
        actT = (a1t, a2t, a3t)
        rowt = sb("rowt", [128, 2, ROWE[0]], bft)
        rowt3 = sb("rowt3", [128, 2, ROWE[2]], bft)
        t1f = sb("t1f", [128, 128], f32)
        m1f = sb("m1f", [128, 128], f32)
        p1f = sb("p1f", [128, 128], f32)
        e1f = sb("e1f", [128, 128], f32)
        tf = sb("tf", [128, 256], f32)
        fm = sb("fm", [128, 256], f32)
        fp = sb("fp", [128, 256], f32)
        fe = sb("fe", [128, 256], f32)
        rcp = sb("rcp", [128, 4], f32)
        aout = sb("aout", [128, 2, 256], bft)
        outr = sb("outr", [128, 2, 128], f32)

        psA = psumt("psA", [128, 2, 512], f32)
        psW = psumt("psW", [128, 4, 512], f32)
        psT = psumt("psT", [128, 2, 128], bft)
        psP = psumt("psP", [128, 2, 128], f32)

        sems = {s: ctx.enter_context(nc.semaphore(f"s_{s}"))
                for s in ("vself", "aself", "tself", "sself", "cc")}
        gsem = [ctx.enter_context(nc.semaphore(f"gq{i}")) for i in range(NGBUF)]
        gcnt = [0] * NGBUF

        # ---------------- boot ----------------
        loads = [(idx_sb[:, :], idxs[:, :]), (pw_sb[:, :], pw[:, :]),
                 (x_sb[:, :], x_own[:, :]), (pb_sb[:, :], pb_rep[:, :]),
                 (w1x_sb[:, :], w1x[:, :]),
                 (w2x_sb[:, :, :], w2x.rearrange("a b c -> b a c")),
                 (w3x_sb[:, :, :], w3x.rearrange("a b c -> b a c")),
                 (b_sb[0][:, :], b1_rep[:, :]), (b_sb[1][:, :], b2_rep[:, :]),
                 (b_sb[2][:, :], b3_rep[:, :]), (ident[:, :], identd[:, :]),
                 (prow1[:, :], padrow1[:, :]), (prow3[:, :], padrow3[:, :])]
        for dst, srcap in loads:
            T["boot_loads"] = plan.add(
                "sync", lambda e, d=dst, s=srcap: e.dma_start(out=d, in_=s))
        for tb, pr, ne in ((table1, prow1, ROWE[0]), (table2, prow1, ROWE[1]),
                           (table3, prow3, ROWE[2])):
            T["boot"] = plan.add(
                "sync",
                lambda e, t=tb, p=pr, ne=ne: e.dma_start(out=t[NV:NV + 1, 0:ne],
                                                         in_=p[0:1, 0:ne]),
                waits=[("sself", T["boot_loads"])])
        BOOTS = T["boot"]
        plan.add("vector", lambda e: e.memset(rowt[:, :, :], 0.0))
        T["rowz"] = plan.add("vector", lambda e: e.memset(rowt3[:, :, :], 0.0))

        # ---------------- stage A ----------------
        def stageA_chunk(l, j):
            ck = l * NWIN + j
            rows = slice(j * 128, (j + 1) * 128)
            ncols = (264, 264, 130)[l]
            rowtile = rowt if l < 2 else rowt3
            rowe = ROWE[l]
            psum = psA[:, j % 2, 0:ncols]
            if l == 0:
                T[("p1", j)] = plan.add(
                    "tensor",
                    lambda e: e.matmul(psP[:, j % 2, :], x_sb[0:1, rows],
                                       pw_sb[0:1, :], start=True, stop=True),
                    waits=[("sself", BOOTS), ("vself", G(("e1a", j - 2)))])
                plan.add(
                    "vector",
                    lambda e: e.tensor_tensor(out=t1f[:, :], in0=psP[:, j % 2, :],
                                              in1=pb_sb[:, :], op=Alu.add),
                    waits=[("sself", BOOTS), ("tself", T[("p1", j)]),
                           ("aself", G(("x1", j - 1)))])
                plan.add(
                    "vector",
                    lambda e: e.tensor_scalar(out=m1f[:, :], in0=t1f[:, :],
                                              scalar1=0.0, scalar2=None, op0=Alu.min))
                T[("e1a", j)] = plan.add(
                    "vector",
                    lambda e: e.tensor_scalar(out=p1f[:, :], in0=t1f[:, :],
                                              scalar1=0.0, scalar2=-1.0,
                                              op0=Alu.max, op1=Alu.add))
                T[("x1", j)] = plan.add(
                    "scalar",
                    lambda e: e.activation(e1f[:, :], m1f[:, :], Act.Exp),
                    waits=[("vself", T[("e1a", j)]), ("vself", G(("e1", j - 1)))])
                T[("e1", j)] = plan.add(
                    "vector",
                    lambda e: e.tensor_tensor(out=act1[:, j % 2, :], in0=e1f[:, :],
                                              in1=p1f[:, :], op=Alu.add),
                    waits=[("aself", T[("x1", j)]), ("tself", G(("t1", j - 2)))])
                T[("t1", j)] = plan.add(
                    "tensor",
                    lambda e: e.transpose(psT[:, j % 2, :], act1[:, j % 2, :],
                                          ident[:, :]),
                    waits=[("vself", T[("e1", j)]), ("aself", G(("c1", j - 2)))])
                T[("c1", j)] = plan.add(
                    "scalar",
                    lambda e: e.activation(a1t[:, rows], psT[:, j % 2, :], Act.Copy),
                    waits=[("tself", T[("t1", j)])])
                T[("feat", ck)] = plan.add(
                    "tensor",
                    lambda e: e.matmul(psum, a1t[:, rows], w1x_sb[:, :],
                                       start=True, stop=True),
                    waits=[("aself", T[("c1", j)]),
                           ("aself", G(("rowcp", ck - 2))),
                           ("vself", G(("adst", ck - 2)))])
            else:
                at = actT[l]
                wsb = (None, w2x_sb, w3x_sb)[l]
                for k in range(2):
                    tk = plan.add(
                        "tensor",
                        lambda e, k=k, at=at, wsb=wsb, psum=psum, rows=rows,
                        ncols=ncols:
                        e.matmul(psum, at[:, k, rows], wsb[:, k, 0:ncols],
                                 start=(k == 0), stop=(k == 1)),
                        waits=([("aself", T[("atc", (l - 1) * NWIN + j)]),
                                ("aself", G(("rowcp", ck - 2))),
                                ("vself", G(("adst", ck - 2)))]
                               if k == 0 else []))
                T[("feat", ck)] = tk
            rw = [("tself", T[("feat", ck)]), ("sself", G(("row", ck - 2)))]
            if ck == 0:
                rw.append(("vself", T["rowz"]))
            T[("rowcp", ck)] = plan.add(
                "scalar",
                lambda e, rowtile=rowtile, ck=ck, ncols=ncols, psum=psum:
                e.activation(rowtile[:, ck % 2, 0:ncols], psum, Act.Copy),
                waits=rw)
            nh = NH[l]
            ac0 = (260, 260, 129)[l]
            T[("adst", ck)] = plan.add(
                "vector",
                lambda e, l=l, j=j, nh=nh, ac0=ac0:
                e.tensor_copy(out=adst_sb[l][:, j, 0:nh],
                              in_=psA[:, j % 2, ac0:ac0 + nh]),
                waits=[("tself", T[("feat", ck)])])
            bn = bounces[l]
            dw = [("aself", T[("rowcp", ck)])]
            if j == 0 and l >= 1:
                dw.append(("cc", l))
            T[("row", ck)] = plan.add(
                "sync",
                lambda e, bn=bn, rows=rows, rowtile=rowtile, ck=ck, rowe=rowe:
                e.dma_start(out=bn[rows, :], in_=rowtile[:, ck % 2, 0:rowe]),
                waits=dw)

        def allgather(l):
            plan.add("gpsimd",
                     lambda e, tb=tables[l], bn=bounces[l]:
                     e.collective_compute("AllGather", Alu.bypass,
                                          replica_groups=[list(range(NCORES))],
                                          ins=[bn[:, :]], outs=[tb[0:NV, :]]),
                     waits=[("sself", T[("row", l * NWIN + NWIN - 1)])],
                     extra_inc=("cc", 1))

        # ---------------- edge sweep ----------------
        qglob = [0]

        def edge_window(l, j):
            rowe = ROWE[l]
            nh = NH[l]
            ch = CH[l]
            rhsc = (260, 260, 129)[l]
            asc0 = (256, 256, 128)[l]
            gt = gA if l < 2 else g3
            wg = l * NWIN + j
            for (s0, bs) in wblocks[j]:
                gq = qglob[0]
                qglob[0] += 1
                slot = gq % NGBUF
                gtile = gt[:, slot]
                colbase = (mbase[j] + s0) * 8
                gwaits = [("cc", l + 1)]
                if gcnt[slot] > 0:
                    gwaits.append(("tself", G(("mm", gq - NGBUF))))
                gcnt[slot] += 1
                need = 16 * gcnt[slot]
                plan.add("gpsimd",
                         lambda e, gtile=gtile, colbase=colbase, bs=bs, slot=slot,
                         rowe=rowe, tb=tables[l], gq=gq:
                         e.dma_gather(gtile[:, 0:bs, :], tb[:, :],
                                      idx_sb[:, colbase:colbase + bs * 8],
                                      bs * 128, bs * 128, rowe,
                                      single_packet=False, queue_num=gq % 4)
                         .then_inc(gsem[slot], 16),
                         waits=gwaits)
                lsl = lambda t, gq=gq, bs=bs, nh=nh: t[:, gq % 2, 0:bs, 0:nh]
                adst_ap = adst_sb[l][:, j:j + 1, 0:nh].to_broadcast([128, bs, nh])
                plan.add("vector",
                         lambda e, gtile=gtile, bs=bs, nh=nh, asc0=asc0, lsl=lsl,
                         adst_ap=adst_ap:
                         e.tensor_tensor(out=lsl(lgt),
                                         in0=gtile[:, 0:bs, asc0:asc0 + nh],
                                         in1=adst_ap, op=Alu.add),
                         waits=[(gsem[slot], need), ("aself", G(("ea", gq - 2))),
                                ("vself", G(("adstv", (l, j))))])
                T[("lg", gq)] = plan.n("vself")
                T[("ea", gq)] = plan.add(
                    "scalar",
                    lambda e, lsl=lsl: e.activation(lsl(ltA), lsl(lgt), Act.Exp),
                    waits=[("vself", T[("lg", gq)]),
                           ("vself", G(("wmax", gq - 2)))])
                T[("eb", gq)] = plan.add(
                    "scalar",
                    lambda e, lsl=lsl: e.activation(lsl(ltB), lsl(lgt), Act.Exp,
                                                    scale=0.2))
                T[("wmax", gq)] = plan.add(
                    "vector",
                    lambda e, gtile=gtile, bs=bs, nh=nh, asc0=asc0, lsl=lsl:
                    e.tensor_tensor(out=gtile[:, 0:bs, asc0:asc0 + nh],
                                    in0=lsl(ltA), in1=lsl(ltB), op=Alu.max),
                    waits=[("aself", T[("eb", gq)])])
                T[("gm", gq)] = plan.add(
                    "vector",
                    lambda e, gtile=gtile, bs=bs, nh=nh, ch=ch, asc0=asc0:
                    e.tensor_tensor(
                        out=gtile[:, 0:bs, 0:ch].rearrange(
                            "p b (h c) -> p b h c", h=nh),
                        in0=gtile[:, 0:bs, 0:ch].rearrange(
                            "p b (h c) -> p b h c", h=nh),
                        in1=gtile[:, 0:bs, asc0:asc0 + nh][:, :, :, None]
                        .to_broadcast([128, bs, nh, ch // nh]),
                        op=Alu.mult))
                for k in range(bs):
                    mw = []
                    if k == 0:
                        mw = [("vself", T[("gm", gq)])]
                        if s0 == 0:
                            mw.append(("vself", G(("fin", wg - 4))))
                    tk = plan.add(
                        "tensor",
                        lambda e, gtile=gtile, k=k, j=j, s0=s0, rhsc=rhsc, mj=m[j]:
                        e.matmul(psW[:, j % 4, 0:rhsc], ident[:, :],
                                 gtile[:, k, 0:rhsc],
                                 start=(s0 + k == 0), stop=(s0 + k == mj - 1)),
                        waits=mw)
                T[("mm", gq)] = tk
            T[("wlast", l, j)] = qglob[0] - 1

        # ---------------- finalize ----------------
        def finalize(l, j):
            wg = l * NWIN + j
            nh = NH[l]
            ch = CH[l]
            dn0 = (256, 256, 128)[l]
            lastq = T[("wlast", l, j)]
            plan.add("vector",
                     lambda e, j=j, nh=nh, dn0=dn0:
                     e.reciprocal(rcp[:, 0:nh], psW[:, j % 4, dn0:dn0 + nh]),
                     waits=[("tself", T[("mm", lastq)])])
            if l < 2:
                T[("fin", wg)] = plan.add(
                    "vector",
                    lambda e, j=j, nh=nh, ch=ch:
                    e.tensor_tensor(
                        out=tf[:, :].rearrange("p (h c) -> p h c", h=nh),
                        in0=psW[:, j % 4, 0:ch].rearrange("p (h c) -> p h c", h=nh),
                        in1=rcp[:, 0:nh][:, :, None].to_broadcast([128, nh, ch // nh]),
                        op=Alu.mult))
                T[("tfb", wg)] = plan.add(
                    "vector",
                    lambda e, l=l: e.tensor_tensor(out=tf[:, :], in0=tf[:, :],
                                                   in1=b_sb[l][:, :], op=Alu.add))
                plan.add("scalar",
                         lambda e: e.activation(fm[:, :], tf[:, :], Act.Relu,
                                                scale=-1.0),
                         waits=[("vself", T[("tfb", wg)]),
                                ("vself", G(("aoutv", wg - 1)))])
                plan.add("scalar",
                         lambda e: e.activation(fe[:, :], fm[:, :], Act.Exp,
                                                scale=-1.0))
                T[("felu", wg)] = plan.add(
                    "scalar",
                    lambda e: e.activation(fp[:, :], tf[:, :], Act.Relu))
                plan.add(
                    "vector",
                    lambda e: e.tensor_tensor(out=fm[:, :], in0=fe[:, :],
                                              in1=fp[:, :], op=Alu.add),
                    waits=[("aself", T[("felu", wg)])])
                T[("aoutv", wg)] = plan.add(
                    "vector",
                    lambda e, wg=wg: e.tensor_scalar(out=aout[:, wg % 2, :],
                                                     in0=fm[:, :], scalar1=-1.0,
                                                     scalar2=None, op0=Alu.add),
                    waits=[("tself", G(("tr", wg - 2)))])
                at = actT[l + 1]
                for k in range(2):
                    tk = plan.add(
                        "tensor",
                        lambda e, k=k, wg=wg: e.transpose(
                            psT[:, k, :], aout[:, wg % 2, k * 128:(k + 1) * 128],
                            ident[:, :]),
                        waits=([("vself", T[("aoutv", wg)]),
                                ("aself", G(("atc", wg - 1)))] if k == 0 else []))
                T[("tr", wg)] = tk
                for k in range(2):
                    tk = plan.add(
                        "scalar",
                        lambda e, k=k, at=at, j=j: e.activation(
                            at[:, k, j * 128:(j + 1) * 128], psT[:, k, :], Act.Copy),
                        waits=([("tself", T[("tr", wg)])] if k == 0 else []))
                T[("atc", wg)] = tk
            else:
                T[("fin", wg)] = plan.add(
                    "vector",
                    lambda e, j=j: e.tensor_tensor(
                        out=outr[:, j % 2, :], in0=psW[:, j % 4, 0:128],
                        in1=rcp[:, 0:1].to_broadcast([128, 128]), op=Alu.mult),
                    waits=[("sself", G(("outd", j - 2)))])
                T[("outrow", j)] = plan.add(
                    "vector",
                    lambda e, j=j: e.tensor_tensor(out=outr[:, j % 2, :],
                                                   in0=outr[:, j % 2, :],
                                                   in1=b_sb[2][:, :], op=Alu.add))
                T[("outd", j)] = plan.add(
                    "sync",
                    lambda e, j=j: e.dma_start(out=out_d[j * 128:(j + 1) * 128, :],
                                               in_=outr[:, j % 2, :]),
                    waits=[("vself", T[("outrow", j)])])

        # ---------------- schedule ----------------
        for j in range(NWIN):
            stageA_chunk(0, j)
        allgather(0)
        for l in range(3):
            for j in range(NWIN):
                # adst for (l, j) was produced by stage A chunk (l, j)
                T[("adstv", (l, j))] = T[("adst", l * NWIN + j)]
                edge_window(l, j)
                finalize(l, j)
                if l < 2:
                    stageA_chunk(l + 1, j)
            if l < 2:
                allgather(l + 1)

        # ---------------- emit ----------------
        def run_engine(eng_obj, name):
            for fn, waits, incs in plan.ops[name]:
                for semname, cnt in waits:
                    h = sems[semname] if isinstance(semname, str) else semname
                    eng_obj.wait_ge(h, cnt)
                instr = fn(eng_obj)
                for semname, cnt in incs:
                    h = sems[semname] if isinstance(semname, str) else semname
                    instr = instr.then_inc(h, cnt)

        with nc.Block() as block:
            @block.gpsimd
            def _(gpsimd):
                gpsimd.load_library(mlp)
                run_engine(gpsimd, "gpsimd")

            @block.sync
            def _(sync):
                run_engine(sync, "sync")

            @block.tensor
            def _(tensor):
                run_engine(tensor, "tensor")

            @block.vector
            def _(vector):
                run_engine(vector, "vector")

            @block.scalar
            def _(scalar):
                run_engine(scalar, "scalar")

        nc.compile()
    return nc


# ---------------------------------------------------------------- entry
def kernel(**inputs):
    from concourse.bass_utils import run_bass_kernel_spmd

    pid_of_node, m, in_maps = _prep_inputs(inputs)
    key = tuple(int(v) for v in m)
    if key not in _CACHE:
        _CACHE[key] = build_program(m)
    nc = _CACHE[key]
    res = run_bass_kernel_spmd(nc, in_maps, list(range(NCORES)))
    outp = np.concatenate([res.results[c]["out"] for c in range(NCORES)], axis=0)
    return outp[pid_of_node].astype(np.float32)



# revision 4
# speedup vs baseline: 1.2683x; 1.0024x over previous
"""Trainium2 Bass kernel for nn_GATMissingEmbedder (3-layer GAT, N=10000, E=320000).

SPMD across 8 NeuronCores. Host relabels nodes into degree-homogeneous
windows of 128 (same per-window slot count on every core). Edge slot
(window j, slot s, partition p) holds the s-th incoming edge of window
node p; holes point at a pad table row whose alpha_src is -200 (weight
exp(lrelu(-200+adst)) ~ e^-40 ~ 0, h-part zero).

Per layer: sharded stage A computes feat rows [h | a_src | a_dst | pad]
(h = act @ W; alpha columns folded into the weight matrix on the host;
biases are applied after aggregation, matching PyG GATConv). AllGather
builds the full table; dma_gather (4 SWDGE queues) pulls per-edge rows
slot-major so partition == destination; DVE/ACT compute
w = exp(leakyrelu(a_src[src] + a_dst[dst])) and scale the gathered h in
place; the segment softmax numerator/denominator accumulate in PSUM via
identity-weight matmuls; window finalize divides, adds bias, applies
ELU, and transposes activations for the next layer's stage A.
"""
import math
from contextlib import ExitStack

import ml_dtypes
import numpy as np

N = 10000
E = 320000
NCORES = 8
P = 128
NWIN = 10                  # windows per core
NDEV = NWIN * P            # 1280 owned (permuted) nodes per core
NV = NCORES * NDEV         # 10240 total permuted node slots
PADROW = NV                # table row used by hole slots
BMAX = 8                   # max slots per gather block
NGBUF = 16                 # gather tile buffers
CH = (256, 256, 128)       # h width per layer
NH = (4, 4, 1)             # heads per layer
ROWE = (384, 384, 256)     # table row elems (bf16) per layer
bf16 = ml_dtypes.bfloat16

_CACHE = {}


# ---------------------------------------------------------------- host prep
def _ablk(a):
    """[H, C] -> block-diagonal [H*C, H]."""
    H, C = a.shape
    out = np.zeros((H * C, H), np.float32)
    for h in range(H):
        out[h * C:(h + 1) * C, h] = a[h]
    return out


def _prep_graph(edge_index):
    src, dst = np.asarray(edge_index[0]), np.asarray(edge_index[1])
    deg = np.bincount(dst, minlength=N)
    order = np.argsort(-deg, kind="stable")          # rank -> orig node
    pid_of_node = np.full(N, -1, np.int64)
    ranks = np.arange(NV)
    rows = ranks // (P * NCORES)
    qs = ranks % (P * NCORES)
    cores = qs % NCORES
    ps = qs // NCORES
    pids = cores * NDEV + rows * P + ps
    real = ranks < N
    pid_of_node[order[ranks[real]]] = pids[real]
    m = np.zeros(NWIN, np.int64)
    for j in range(NWIN):
        rj = ranks[(rows == j) & real]
        m[j] = max(int(deg[order[rj]].max()) if len(rj) else 0, 1)
    pdst = pid_of_node[dst]
    psrc = pid_of_node[src]
    eorder = np.argsort(pdst, kind="stable")
    pdst_s, psrc_s = pdst[eorder], psrc[eorder]
    starts = np.searchsorted(pdst_s, np.arange(NV))
    ends = np.searchsorted(pdst_s, np.arange(NV) + 1)
    nslot = int(m.sum())
    idx_all = np.full((NCORES, nslot * P), PADROW, np.int64)
    base = np.concatenate([[0], np.cumsum(m)])
    for c in range(NCORES):
        for j in range(NWIN):
            for p in range(P):
                pid = c * NDEV + j * P + p
                s0, s1 = starts[pid], ends[pid]
                if s1 == s0:
                    continue
                sl = (base[j] + np.arange(s1 - s0)) * P + p
                idx_all[c, sl] = psrc_s[s0:s1]
    ni = nslot * P
    wrapped = np.zeros((NCORES, 128, ni // 16), np.int16)
    for c in range(NCORES):
        w16 = idx_all[c].reshape(ni // 16, 16).T.astype(np.int16)
        wrapped[c] = np.tile(w16, (8, 1))
    return pid_of_node, m, wrapped


def _prep_inputs(inputs):
    pid_of_node, m, wrapped = _prep_graph(inputs["edge_index"])
    w1 = np.asarray(inputs["w1"], np.float32)
    w2 = np.asarray(inputs["w2"], np.float32)
    w3 = np.asarray(inputs["w3"], np.float32)
    wx1 = np.concatenate(
        [w1, w1 @ _ablk(np.asarray(inputs["a_src1"], np.float32)),
         w1 @ _ablk(np.asarray(inputs["a_dst1"], np.float32))], axis=1)
    wx2 = np.concatenate(
        [w2, w2 @ _ablk(np.asarray(inputs["a_src2"], np.float32)),
         w2 @ _ablk(np.asarray(inputs["a_dst2"], np.float32))], axis=1)
    wx3 = np.concatenate(
        [w3, w3 @ _ablk(np.asarray(inputs["a_src3"], np.float32)),
         w3 @ _ablk(np.asarray(inputs["a_dst3"], np.float32))], axis=1)
    x = np.asarray(inputs["x"], np.float32).reshape(-1)
    xp = np.zeros(NV, np.float32)
    xp[pid_of_node] = x
    padrow1 = np.zeros((1, ROWE[0]), bf16)
    padrow1[0, 256:260] = bf16(-200.0)
    padrow3 = np.zeros((1, ROWE[2]), bf16)
    padrow3[0, 128] = bf16(-200.0)
    common = {
        "pw": np.asarray(inputs["proj_w"], np.float32).reshape(1, 128),
        "pb_rep": np.tile(np.asarray(inputs["proj_b"], np.float32).reshape(1, 128), (128, 1)),
        "w1x": wx1.astype(bf16),
        "w2x": wx2.reshape(2, 128, 264).astype(bf16),
        "w3x": wx3.reshape(2, 128, 130).astype(bf16),
        "b1_rep": np.tile(np.asarray(inputs["b1"], np.float32).reshape(1, 256), (128, 1)),
        "b2_rep": np.tile(np.asarray(inputs["b2"], np.float32).reshape(1, 256), (128, 1)),
        "b3_rep": np.tile(np.asarray(inputs["b3"], np.float32).reshape(1, 128), (128, 1)),
        "ident": np.eye(128, dtype=bf16),
        "padrow1": padrow1,
        "padrow3": padrow3,
    }
    in_maps = []
    for c in range(NCORES):
        d = dict(common)
        d["x_own"] = xp[c * NDEV:(c + 1) * NDEV].reshape(1, NDEV).copy()
        d["idxs"] = wrapped[c]
        in_maps.append(d)
    return pid_of_node, m, in_maps


# ---------------------------------------------------------------- plan
class Plan:
    """Per-engine op lists. One counting semaphore per engine: every op
    increments its engine's sem (DMA ops by 16), and cross/same-engine
    dependencies wait on recorded ordinals. This matches the in-order
    engines and keeps one update per instruction."""

    ENGSEM = {"vector": "vself", "scalar": "aself", "tensor": "tself",
              "sync": "sself"}
    STEP = {"vector": 1, "scalar": 1, "tensor": 1, "sync": 16}

    def __init__(self):
        self.ops = {e: [] for e in ("sync", "gpsimd", "tensor", "vector", "scalar")}
        self.count = {}

    def add(self, eng, fn, waits=(), extra_inc=None):
        """Returns the engine-sem value after this op completes."""
        incs = []
        after = None
        if eng in self.ENGSEM:
            s = self.ENGSEM[eng]
            step = self.STEP[eng]
            prev = self.count.get(s, 0)
            waits = list(waits)
            if eng in ("vector", "scalar"):
                waits.append((s, prev))          # same-engine in-order model
            incs.append((s, step))
            after = prev + step
        if extra_inc is not None:
            incs.append(extra_inc)
        self.ops[eng].append((fn, [w for w in waits if w[1] > 0], incs))
        for sem, k in incs:
            self.count[sem] = self.count.get(sem, 0) + k
        return after

    def n(self, sem):
        return self.count.get(sem, 0)


# ---------------------------------------------------------------- program
def build_program(m):
    import concourse.bacc as bacc
    import concourse.mybir as mybir
    from concourse.library_config import mlp

    f32, bft, i16 = mybir.dt.float32, mybir.dt.bfloat16, mybir.dt.int16
    Alu = mybir.AluOpType
    Act = mybir.ActivationFunctionType

    m = [int(v) for v in m]
    nslot = sum(m)
    ni = nslot * P
    nc = bacc.Bacc("TRN2", num_swdge_queues=4)

    x_own = nc.dram_tensor("x_own", [1, NDEV], f32, kind="ExternalInput")
    idxs = nc.dram_tensor("idxs", [128, ni // 16], i16, kind="ExternalInput")
    pw = nc.dram_tensor("pw", [1, 128], f32, kind="ExternalInput")
    pb_rep = nc.dram_tensor("pb_rep", [128, 128], f32, kind="ExternalInput")
    w1x = nc.dram_tensor("w1x", [128, 264], bft, kind="ExternalInput")
    w2x = nc.dram_tensor("w2x", [2, 128, 264], bft, kind="ExternalInput")
    w3x = nc.dram_tensor("w3x", [2, 128, 130], bft, kind="ExternalInput")
    b1_rep = nc.dram_tensor("b1_rep", [128, 256], f32, kind="ExternalInput")
    b2_rep = nc.dram_tensor("b2_rep", [128, 256], f32, kind="ExternalInput")
    b3_rep = nc.dram_tensor("b3_rep", [128, 128], f32, kind="ExternalInput")
    identd = nc.dram_tensor("ident", [128, 128], bft, kind="ExternalInput")
    padrow1 = nc.dram_tensor("padrow1", [1, ROWE[0]], bft, kind="ExternalInput")
    padrow3 = nc.dram_tensor("padrow3", [1, ROWE[2]], bft, kind="ExternalInput")
    out_d = nc.dram_tensor("out", [NDEV, 128], f32, kind="ExternalOutput")

    table1 = nc.dram_tensor("table1", [NV + 1, ROWE[0]], bft, addr_space="Shared")
    table2 = nc.dram_tensor("table2", [NV + 1, ROWE[1]], bft, addr_space="Shared")
    table3 = nc.dram_tensor("table3", [NV + 1, ROWE[2]], bft, addr_space="Shared")
    bounce = nc.dram_tensor("bounce", [NDEV, ROWE[0]], bft)
    bounce3 = nc.dram_tensor("bounce3", [NDEV, ROWE[2]], bft)
    tables = (table1, table2, table3)
    bounces = (bounce, bounce, bounce3)

    wblocks = []
    for j in range(NWIN):
        bl, s0 = [], 0
        while s0 < m[j]:
            bs = min(BMAX, m[j] - s0)
            bl.append((s0, bs))
            s0 += bs
        wblocks.append(bl)
    mbase = [0]
    for j in range(NWIN):
        mbase.append(mbase[-1] + m[j])

    plan = Plan()
    T = {}          # recorded ordinals: T[(kind, idx)] = engine-sem threshold
    G = lambda k, d=0: T.get(k, d)

    with ExitStack() as ctx:
        sb = lambda name, shape, dt: ctx.enter_context(nc.sbuf_tensor(name, shape, dt))
        psumt = lambda name, shape, dt: ctx.enter_context(nc.psum_tensor(name, shape, dt))

        idx_sb = sb("idx_sb", [128, ni // 16], i16)
        pw_sb = sb("pw_sb", [1, 128], f32)
        x_sb = sb("x_sb", [1, NDEV], f32)
        pb_sb = sb("pb_sb", [128, 128], f32)
        w1x_sb = sb("w1x_sb", [128, 264], bft)
        w2x_sb = sb("w2x_sb", [128, 2, 264], bft)
        w3x_sb = sb("w3x_sb", [128, 2, 130], bft)
        b_sb = [sb("b1_sb", [128, 256], f32), sb("b2_sb", [128, 256], f32),
                sb("b3_sb", [128, 128], f32)]
        ident = sb("ident_sb", [128, 128], bft)
        prow1 = sb("prow1", [1, ROWE[0]], bft)
        prow3 = sb("prow3", [1, ROWE[2]], bft)

        gA = sb("gA", [128, NGBUF, BMAX, ROWE[0]], bft)
        g3 = sb("g3", [128, NGBUF, BMAX, ROWE[2]], bft)
        lgt = sb("lgt", [128, 2, BMAX, 4], f32)
        ltA = sb("ltA", [128, 2, BMAX, 4], f32)
        ltB = sb("ltB", [128, 2, BMAX, 4], f32)
        adst_sb = [sb(f"adst{l}", [128, NWIN, 4], bft) for l in range(3)]
        act1 = sb("act1", [128, 2, 128], bft)
        a1t = sb("a1t", [128, NDEV], bft)
        a2t = sb("a2t", [128, 2, NDEV], bft)
        a3t = sb("a3t", [128, 2, NDEV], bft)
        actT = (a1t, a2t, a3t)
        rowt = sb("rowt", [128, 2, ROWE[0]], bft)
        rowt3 = sb("rowt3", [128, 2, ROWE[2]], bft)
        t1f = sb("t1f", [128, 128], f32)
        m1f = sb("m1f", [128, 128], f32)
        p1f = sb("p1f", [128, 128], f32)
        e1f = sb("e1f", [128, 128], f32)
        tf = sb("tf", [128, 256], f32)
        fm = sb("fm", [128, 256], f32)
        fp = sb("fp", [128, 256], f32)
        fe = sb("fe", [128, 256], f32)
        rcp = sb("rcp", [128, 4], f32)
        aout = sb("aout", [128, 2, 256], bft)
        outr = sb("outr", [128, 2, 128], f32)

        psA = psumt("psA", [128, 2, 512], f32)
        psW = psumt("psW", [128, 4, 512], f32)
        psT = psumt("psT", [128, 2, 128], bft)
        psP = psumt("psP", [128, 2, 128], f32)

        sems = {s: ctx.enter_context(nc.semaphore(f"s_{s}"))
                for s in ("vself", "aself", "tself", "sself", "cc")}
        gsem = [ctx.enter_context(nc.semaphore(f"gq{i}")) for i in range(NGBUF)]
        gcnt = [0] * NGBUF

        # ---------------- boot ----------------
        loads = [(idx_sb[:, :], idxs[:, :]), (pw_sb[:, :], pw[:, :]),
                 (x_sb[:, :], x_own[:, :]), (pb_sb[:, :], pb_rep[:, :]),
                 (w1x_sb[:, :], w1x[:, :]),
                 (w2x_sb[:, :, :], w2x.rearrange("a b c -> b a c")),
                 (w3x_sb[:, :, :], w3x.rearrange("a b c -> b a c")),
                 (b_sb[0][:, :], b1_rep[:, :]), (b_sb[1][:, :], b2_rep[:, :]),
                 (b_sb[2][:, :], b3_rep[:, :]), (ident[:, :], identd[:, :]),
                 (prow1[:, :], padrow1[:, :]), (prow3[:, :], padrow3[:, :])]
        for dst, srcap in loads:
            T["boot_loads"] = plan.add(
                "sync", lambda e, d=dst, s=srcap: e.dma_start(out=d, in_=s))
        for tb, pr, ne in ((table1, prow1, ROWE[0]), (table2, prow1, ROWE[1]),
                           (table3, prow3, ROWE[2])):
            T["boot"] = plan.add(
                "sync",
                lambda e, t=tb, p=pr, ne=ne: e.dma_start(out=t[NV:NV + 1, 0:ne],
                                                         in_=p[0:1, 0:ne]),
                waits=[("sself", T["boot_loads"])])
        BOOTS = T["boot"]
        plan.add("vector", lambda e: e.memset(rowt[:, :, :], 0.0))
        T["rowz"] = plan.add("vector", lambda e: e.memset(rowt3[:, :, :], 0.0))

        # ---------------- stage A ----------------
        def stageA_chunk(l, j):
            ck = l * NWIN + j
            rows = slice(j * 128, (j + 1) * 128)
            ncols = (264, 264, 130)[l]
            rowtile = rowt if l < 2 else rowt3
            rowe = ROWE[l]
            psum = psA[:, j % 2, 0:ncols]
            if l == 0:
                T[("p1", j)] = plan.add(
                    "tensor",
                    lambda e: e.matmul(psP[:, j % 2, :], x_sb[0:1, rows],
                                       pw_sb[0:1, :], start=True, stop=True),
                    waits=[("sself", BOOTS), ("vself", G(("e1a", j - 2)))])
                plan.add(
                    "vector",
                    lambda e: e.tensor_tensor(out=t1f[:, :], in0=psP[:, j % 2, :],
                                              in1=pb_sb[:, :], op=Alu.add),
                    waits=[("sself", BOOTS), ("tself", T[("p1", j)]),
                           ("aself", G(("x1", j - 1)))])
                plan.add(
                    "vector",
                    lambda e: e.tensor_scalar(out=m1f[:, :], in0=t1f[:, :],
                                              scalar1=0.0, scalar2=None, op0=Alu.min))
                T[("e1a", j)] = plan.add(
                    "vector",
                    lambda e: e.tensor_scalar(out=p1f[:, :], in0=t1f[:, :],
                                              scalar1=0.0, scalar2=-1.0,
                                              op0=Alu.max, op1=Alu.add))
                T[("x1", j)] = plan.add(
                    "scalar",
                    lambda e: e.activation(e1f[:, :], m1f[:, :], Act.Exp),
                    waits=[("vself", T[("e1a", j)]), ("vself", G(("e1", j - 1)))])
                T[("e1", j)] = plan.add(
                    "vector",
                    lambda e: e.tensor_tensor(out=act1[:, j % 2, :], in0=e1f[:, :],
                                              in1=p1f[:, :], op=Alu.add),
                    waits=[("aself", T[("x1", j)]), ("tself", G(("t1", j - 2)))])
                T[("t1", j)] = plan.add(
                    "tensor",
                    lambda e: e.transpose(psT[:, j % 2, :], act1[:, j % 2, :],
                                          ident[:, :]),
                    waits=[("vself", T[("e1", j)]), ("aself", G(("c1", j - 2)))])
                T[("c1", j)] = plan.add(
                    "scalar",
                    lambda e: e.activation(a1t[:, rows], psT[:, j % 2, :], Act.Copy),
                    waits=[("tself", T[("t1", j)])])
                T[("feat", ck)] = plan.add(
                    "tensor",
                    lambda e: e.matmul(psum, a1t[:, rows], w1x_sb[:, :],
                                       start=True, stop=True),
                    waits=[("aself", T[("c1", j)]),
                           ("aself", G(("rowcp", ck - 2))),
                           ("vself", G(("adst", ck - 2)))])
            else:
                at = actT[l]
                wsb = (None, w2x_sb, w3x_sb)[l]
                for k in range(2):
                    tk = plan.add(
                        "tensor",
                        lambda e, k=k, at=at, wsb=wsb, psum=psum, rows=rows,
                        ncols=ncols:
                        e.matmul(psum, at[:, k, rows], wsb[:, k, 0:ncols],
                                 start=(k == 0), stop=(k == 1)),
                        waits=([("aself", T[("atc", (l - 1) * NWIN + j)]),
                                ("aself", G(("rowcp", ck - 2))),
                                ("vself", G(("adst", ck - 2)))]
                               if k == 0 else []))
                T[("feat", ck)] = tk
            rw = [("tself", T[("feat", ck)]), ("sself", G(("row", ck - 2)))]
            if ck == 0:
                rw.append(("vself", T["rowz"]))
            T[("rowcp", ck)] = plan.add(
                "scalar",
                lambda e, rowtile=rowtile, ck=ck, ncols=ncols, psum=psum:
                e.activation(rowtile[:, ck % 2, 0:ncols], psum, Act.Copy),
                waits=rw)
            nh = NH[l]
            ac0 = (260, 260, 129)[l]
            T[("adst", ck)] = plan.add(
                "vector",
                lambda e, l=l, j=j, nh=nh, ac0=ac0:
                e.tensor_copy(out=adst_sb[l][:, j, 0:nh],
                              in_=psA[:, j % 2, ac0:ac0 + nh]),
                waits=[("tself", T[("feat", ck)])])
            bn = bounces[l]
            dw = [("aself", T[("rowcp", ck)])]
            if j == 0 and l >= 1:
                dw.append(("cc", l))
            T[("row", ck)] = plan.add(
                "sync",
                lambda e, bn=bn, rows=rows, rowtile=rowtile, ck=ck, rowe=rowe:
                e.dma_start(out=bn[rows, :], in_=rowtile[:, ck % 2, 0:rowe]),
                waits=dw)

        def allgather(l):
            plan.add("gpsimd",
                     lambda e, tb=tables[l], bn=bounces[l]:
                     e.collective_compute("AllGather", Alu.bypass,
                                          replica_groups=[list(range(NCORES))],
                                          ins=[bn[:, :]], outs=[tb[0:NV, :]]),
                     waits=[("sself", T[("row", l * NWIN + NWIN - 1)])],
                     extra_inc=("cc", 1))

        # ---------------- edge sweep ----------------
        qglob = [0]

        def edge_window(l, j):
            rowe = ROWE[l]
            nh = NH[l]
            ch = CH[l]
            rhsc = (260, 260, 129)[l]
            asc0 = (256, 256, 128)[l]
            gt = gA if l < 2 else g3
            wg = l * NWIN + j
            for (s0, bs) in wblocks[j]:
                gq = qglob[0]
                qglob[0] += 1
                slot = gq % NGBUF
                gtile = gt[:, slot]
                colbase = (mbase[j] + s0) * 8
                gwaits = [("cc", l + 1)]
                if gcnt[slot] > 0:
                    gwaits.append(("tself", G(("mm", gq - NGBUF))))
                gcnt[slot] += 1
                need = 16 * gcnt[slot]
                plan.add("gpsimd",
                         lambda e, gtile=gtile, colbase=colbase, bs=bs, slot=slot,
                         rowe=rowe, tb=tables[l], gq=gq:
                         e.dma_gather(gtile[:, 0:bs, :], tb[:, :],
                                      idx_sb[:, colbase:colbase + bs * 8],
                                      bs * 128, bs * 128, rowe,
                                      single_packet=False, queue_num=gq % 4)
                         .then_inc(gsem[slot], 16),
                         waits=gwaits)
                lsl = lambda t, gq=gq, bs=bs, nh=nh: t[:, gq % 2, 0:bs, 0:nh]
                adst_ap = adst_sb[l][:, j:j + 1, 0:nh].to_broadcast([128, bs, nh])
                plan.add("vector",
                         lambda e, gtile=gtile, bs=bs, nh=nh, asc0=asc0, lsl=lsl,
                         adst_ap=adst_ap:
                         e.tensor_tensor(out=lsl(lgt),
                                         in0=gtile[:, 0:bs, asc0:asc0 + nh],
                                         in1=adst_ap, op=Alu.add),
                         waits=[(gsem[slot], need), ("aself", G(("ea", gq - 2))),
                                ("vself", G(("adstv", (l, j))))])
                T[("lg", gq)] = plan.n("vself")
                T[("ea", gq)] = plan.add(
                    "scalar",
                    lambda e, lsl=lsl: e.activation(lsl(ltA), lsl(lgt), Act.Exp),
                    waits=[("vself", T[("lg", gq)]),
                           ("vself", G(("wmax", gq - 2)))])
                T[("eb", gq)] = plan.add(
                    "scalar",
                    lambda e, lsl=lsl: e.activation(lsl(ltB), lsl(lgt), Act.Exp,
                                                    scale=0.2))
                T[("wmax", gq)] = plan.add(
                    "vector",
                    lambda e, gtile=gtile, bs=bs, nh=nh, asc0=asc0, lsl=lsl:
                    e.tensor_tensor(out=gtile[:, 0:bs, asc0:asc0 + nh],
                                    in0=lsl(ltA), in1=lsl(ltB), op=Alu.max),
                    waits=[("aself", T[("eb", gq)])])
                T[("gm", gq)] = plan.add(
                    "vector",
                    lambda e, gtile=gtile, bs=bs, nh=nh, ch=ch, asc0=asc0:
                    e.tensor_tensor(
                        out=gtile[:, 0:bs, 0:ch].rearrange(
                            "p b (h c) -> p b h c", h=nh),
                        in0=gtile[:, 0:bs, 0:ch].rearrange(
                            "p b (h c) -> p b h c", h=nh),
                        in1=gtile[:, 0:bs, asc0:asc0 + nh][:, :, :, None]
                        .to_broadcast([128, bs, nh, ch // nh]),
                        op=Alu.mult))
                for k in range(bs):
                    mw = []
                    if k == 0:
                        mw = [("vself", T[("gm", gq)])]
                        if s0 == 0:
                            mw.append(("vself", G(("fin", wg - 4))))
                    tk = plan.add(
                        "tensor",
                        lambda e, gtile=gtile, k=k, j=j, s0=s0, rhsc=rhsc, mj=m[j]:
                        e.matmul(psW[:, j % 4, 0:rhsc], ident[:, :],
                                 gtile[:, k, 0:rhsc],
                                 start=(s0 + k == 0), stop=(s0 + k == mj - 1)),
                        waits=mw)
                T[("mm", gq)] = tk
            T[("wlast", l, j)] = qglob[0] - 1

        # ---------------- finalize ----------------
        def finalize(l, j):
            wg = l * NWIN + j
            nh = NH[l]
            ch = CH[l]
            dn0 = (256, 256, 128)[l]
            lastq = T[("wlast", l, j)]
            plan.add("vector",
                     lambda e, j=j, nh=nh, dn0=dn0:
                     e.reciprocal(rcp[:, 0:nh], psW[:, j % 4, dn0:dn0 + nh]),
                     waits=[("tself", T[("mm", lastq)])])
            if l < 2:
                T[("fin", wg)] = plan.add(
                    "vector",
                    lambda e, j=j, nh=nh, ch=ch:
                    e.tensor_tensor(
                        out=tf[:, :].rearrange("p (h c) -> p h c", h=nh),
                        in0=psW[:, j % 4, 0:ch].rearrange("p (h c) -> p h c", h=nh),
                        in1=rcp[:, 0:nh][:, :, None].to_broadcast([128, nh, ch // nh]),
                        op=Alu.mult))
                T[("tfb", wg)] = plan.add(
                    "vector",
                    lambda e, l=l: e.tensor_tensor(out=tf[:, :], in0=tf[:, :],
                                                   in1=b_sb[l][:, :], op=Alu.add))
                plan.add("scalar",
                         lambda e: e.activation(fm[:, :], tf[:, :], Act.Relu,
                                                scale=-1.0),
                         waits=[("vself", T[("tfb", wg)]),
                                ("vself", G(("aoutv", wg - 1)))])
                plan.add("scalar",
                         lambda e: e.activation(fe[:, :], fm[:, :], Act.Exp,
                                                scale=-1.0))
                T[("felu", wg)] = plan.add(
                    "scalar",
                    lambda e: e.activation(fp[:, :], tf[:, :], Act.Relu))
                plan.add(
                    "vector",
                    lambda e: e.tensor_tensor(out=fm[:, :], in0=fe[:, :],
                                              in1=fp[:, :], op=Alu.add),
                    waits=[("aself", T[("felu", wg)])])
                T[("aoutv", wg)] = plan.add(
                    "vector",
                    lambda e, wg=wg: e.tensor_scalar(out=aout[:, wg % 2, :],
                                                     in0=fm[:, :], scalar1=-1.0,
                                                     scalar2=None, op0=Alu.add),
                    waits=[("tself", G(("tr", wg - 2)))])
                at = actT[l + 1]
                for k in range(2):
                    tk = plan.add(
                        "tensor",
                        lambda e, k=k, wg=wg: e.transpose(
                            psT[:, k, :], aout[:, wg % 2, k * 128:(k + 1) * 128],
                            ident[:, :]),
                        waits=([("vself", T[("aoutv", wg)]),
                                ("aself", G(("atc", wg - 1)))] if k == 0 else []))
                T[("tr", wg)] = tk
                for k in range(2):
                    tk = plan.add(
                        "scalar",
                        lambda e, k=k, at=at, j=j: e.activation(
                            at[:, k, j * 128:(j + 1) * 128], psT[:, k, :], Act.Copy),
                        waits=([("tself", T[("tr", wg)])] if k == 0 else []))
                T[("atc", wg)] = tk
            else:
                T[("fin", wg)] = plan.add(
                    "vector",
                    lambda e, j=j: e.tensor_tensor(
                        out=outr[:, j % 2, :], in0=psW[:, j % 4, 0:128],
                        in1=rcp[:, 0:1].to_broadcast([128, 128]), op=Alu.mult),
                    waits=[("sself", G(("outd", j - 2)))])
                T[("outrow", j)] = plan.add(
                    "vector",
                    lambda e, j=j: e.tensor_tensor(out=outr[:, j % 2, :],
                                                   in0=outr[:, j % 2, :],
                                                   in1=b_sb[2][:, :], op=Alu.add))
                T[("outd", j)] = plan.add(
                    "sync",
                    lambda e, j=j: e.dma_start(out=out_d[j * 128:(j + 1) * 128, :],
                                               in_=outr[:, j % 2, :]),
                    waits=[("vself", T[("outrow", j)])])

        # ---------------- schedule ----------------
        for j in range(NWIN):
            stageA_chunk(0, j)
        allgather(0)
        for l in range(3):
            for j in range(NWIN):
                # adst for (l, j) was produced by stage A chunk (l, j)
                T[("adstv", (l, j))] = T[("adst", l * NWIN + j)]
                edge_window(l, j)
                finalize(l, j)
                if l < 2:
                    stageA_chunk(l + 1, j)
            if l < 2:
                allgather(l + 1)

        # ---------------- emit ----------------
        def run_engine(eng_obj, name):
            for fn, waits, incs in plan.ops[name]:
                for semname, cnt in waits:
                    h = sems[semname] if isinstance(semname, str) else semname
                    eng_obj.wait_ge(h, cnt)
                instr = fn(eng_obj)
                for semname, cnt in incs:
                    h = sems[semname] if isinstance(semname, str) else semname
                    instr = instr.then_inc(h, cnt)

        with nc.Block(no_gpsimd_drain=True) as block:
            @block.gpsimd
            def _(gpsimd):
                gpsimd.load_library(mlp)
                run_engine(gpsimd, "gpsimd")

            @block.sync
            def _(sync):
                run_engine(sync, "sync")

            @block.tensor
            def _(tensor):
                run_engine(tensor, "tensor")

            @block.vector
            def _(vector):
                run_engine(vector, "vector")

            @block.scalar
            def _(scalar):
                run_engine(scalar, "scalar")

        nc.compile()
    return nc


# ---------------------------------------------------------------- entry
def kernel(**inputs):
    from concourse.bass_utils import run_bass_kernel_spmd

    pid_of_node, m, in_maps = _prep_inputs(inputs)
    key = tuple(int(v) for v in m)
    if key not in _CACHE:
        _CACHE[key] = build_program(m)
    nc = _CACHE[key]
    res = run_bass_kernel_spmd(nc, in_maps, list(range(NCORES)))
    outp = np.concatenate([res.results[c]["out"] for c in range(NCORES)], axis=0)
    return outp[pid_of_node].astype(np.float32)

